# revision 5
# baseline (speedup 1.0000x reference)
"""DGCNN (3x DynamicEdgeConv + global max pool + MLP head) on 8 Trainium2
NeuronCores, data-parallel over the batch (one point cloud per core).

EdgeConv algebra: h_ij = [x_i, x_j - x_i] @ W + b = u_i + v_j with
  u = x @ (Wa - Wb) + b,  v = x @ Wb;  out_i = u_i + max_{j in knn(i)} v_j.

kNN key d''_ij = 2 x_i.x_j - |x_j|^2 (largest = nearest); the self column is
killed with a -1e30 diagonal matmul so the top-20 are exactly the neighbors.
Distances are computed with fp32r matmuls whose contraction is augmented with
a ones/nsq row pair (one matmul per 512-col chunk). The column index is
embedded into the low 11 mantissa bits of each distance (bitwise AND+OR with
an iota row), so top-k selection needs no max_index passes: 16 segment max8's
produce 128 candidates, 3x max8 + 2x match_replace pick the top-24, and the
indices pop out of the winning values with a bitwise AND.
Neighbor v-rows are fetched with 3 batched dma_gather calls (1024+1024+512
rows) whose int16 index list is built by 8 permutation matmuls that transpose
jtab into the gather's wrapped 16-partition layout. The 20-way neighbor max
is a single DVE tensor_reduce, added to u to form the layer output.
"""
import numpy as np

_NC_CACHE = {}

N, NT, JC = 2048, 16, 4


def _builder():
    import concourse.bacc as bacc
    import concourse.mybir as mybir
    from concourse.tile import TileContext

    F32 = mybir.dt.float32
    F32R = mybir.dt.float32r
    F16 = mybir.dt.float16
    U32 = mybir.dt.uint32
    I16 = mybir.dt.int16
    AF = mybir.ActivationFunctionType
    ALU = mybir.AluOpType
    AX = mybir.AxisListType

    def ts(i, s):
        return slice(i * s, (i + 1) * s)

    nc = bacc.Bacc("TRN2", num_devices=8, num_swdge_queues=4)

    def din(name, shape, dt=F32R):
        return nc.dram_tensor(name, shape, dt, kind="ExternalInput").ap()

    hA1 = din("hA1", [5, N])            # [x^T; nsq; ones]
    hX1 = din("hX1", [4, N])            # [2 x^T; ones]
    ABc1 = din("ABc1", [5, 64])
    BB1 = din("BB1", [3, 64])
    ABc2 = din("ABc2", [66, 128])
    BB2 = din("BB2", [64, 128])
    AB3 = din("AB3", [128, 256])
    BB3 = din("BB3", [128, 256])
    b3r = din("b3r", [1, 256])
    idn = din("idn", [128, 128], F32)
    idnN = din("idnN", [128, 128])      # -1e30 * I
    idnP = din("idnP", [128, 128])      # I
    onesr = din("onesr", [1, N])        # ones row
    onescol = din("onescol", [128, 1])  # ones column
    iot_in = din("iot", [128, N], U32)
    prm_in = din("prm", [128, 8, 128], F16)
    fc1w = din("fc1w", [256, 512], F32)
    fc1b = din("fc1b", [128, 4], F32)
    fc2w = din("fc2w", [512, 256], F32)
    fc2b = din("fc2b", [128, 2], F32)
    fc3w = din("fc3w", [256, 16], F32)
    fc3b = din("fc3b", [16, 1], F32)
    out = nc.dram_tensor("out", [16, 1], F32, kind="ExternalOutput").ap()

    def ts_imm(out_ap, in0, imm):
        eng = nc.vector
        return eng.add_instruction(
            mybir.InstTensorScalarPtr(
                name=eng.bass.get_next_instruction_name(),
                op0=ALU.bitwise_and, op1=ALU.bypass,
                ins=[eng.lower_ap(in0),
                     mybir.ImmediateValue(dtype=U32, value=imm)],
                outs=[eng.lower_ap(out_ap)]))

    def stt_imm(out_ap, in0, imm, in1):
        eng = nc.vector
        return eng.add_instruction(
            mybir.InstTensorScalarPtr(
                name=eng.bass.get_next_instruction_name(),
                is_scalar_tensor_tensor=True,
                op0=ALU.bitwise_and, op1=ALU.bitwise_or,
                ins=[eng.lower_ap(in0),
                     mybir.ImmediateValue(dtype=U32, value=imm),
                     eng.lower_ap(in1)],
                outs=[eng.lower_ap(out_ap)]))

    v_drams = {}

    def run_layer(tc, layer, C, D, hA, hX, nsq_l3, ones1, iot, prm, u_cfg, out_h):
        """hA: [C+2, N] (feat, nsq, ones) f32r for L1/L2; [128, N] feat for L3.
        hX: [C+1, N] (2feat, ones) for L1/L2; [128, N] 2feat for L3.
        Returns h [128, NT, D] f32 (i = t*128 + p)."""
        l3 = layer == 3
        vslice = v_drams[D]
        ABc, BB = u_cfg
        with tc.tile_pool(name=f"L{layer}", bufs=1) as lp:
            u = lp.tile([128, NT, D], F32, name=f"u_{layer}")
            # ---------- u/v matmuls, v -> DRAM ----------
            with tc.tile_pool(name=f"L{layer}uv", bufs=4, space="PSUM") as uvps, \
                 tc.tile_pool(name=f"L{layer}uvsb", bufs=4) as uvsb:
                for t in range(NT):
                    vp = uvps.tile([128, D], F32, name="vp")
                    nc.tensor.matmul(vp[:], hA[0:C, ts(t, 128)], BB[:],
                                     start=True, stop=True)
                    up = uvps.tile([128, D], F32, name="up")
                    if l3:
                        nc.tensor.matmul(up[:], hA[:, ts(t, 128)], ABc[:],
                                         start=True, stop=False)
                        nc.tensor.matmul(up[:], ones1, b3r_sb[:],
                                         start=False, stop=True)
                    else:
                        nc.tensor.matmul(up[:], hA[0:C + 2, ts(t, 128)], ABc[:],
                                         start=True, stop=True)
                    vsb = uvsb.tile([128, D], F32, name="vsb")
                    nc.scalar.copy(vsb[:], vp[:])
                    nc.scalar.copy(u[:, t, :], up[:])
                    nc.sync.dma_start(vslice[ts(t, 128), :], vsb[:])

            # ---------- per-tile: dist + select + gather + reduce ----------
            h = out_h if out_h is not None else lp.tile([128, NT, D], F32,
                                                        name=f"h_{layer}")
            with tc.tile_pool(name=f"L{layer}d", bufs=1, space="PSUM") as dps, \
                 tc.tile_pool(name=f"L{layer}w", bufs=2, space="PSUM") as wps, \
                 tc.tile_pool(name=f"L{layer}dd", bufs=2) as ddp, \
                 tc.tile_pool(name=f"L{layer}sel", bufs=2) as selp, \
                 tc.tile_pool(name=f"L{layer}g", bufs=2) as gp:
                for t in range(NT):
                    dp = dps.tile([128, N], F32, name="dp")
                    tchunk = t // 4
                    for j in range(JC):
                        last = (j != tchunk)
                        if l3:
                            nc.tensor.matmul(dp[:, ts(j, 512)], hX[:, ts(t, 128)],
                                             hA[:, ts(j, 512)],
                                             start=True, stop=False)
                            nc.tensor.matmul(dp[:, ts(j, 512)], ones1,
                                             nsq_l3[:, ts(j, 512)],
                                             start=False, stop=last)
                        else:
                            nc.tensor.matmul(dp[:, ts(j, 512)], hX[:, ts(t, 128)],
                                             hA[0:C + 1, ts(j, 512)],
                                             start=True, stop=last)
                        if not last:
                            nc.tensor.matmul(dp[:, ts(t, 128)], idnN_sb[:],
                                             idnP_sb[:], start=False, stop=True,
                                             skip_group_check=True)
                    dde = ddp.tile([128, N], F32, name="dde")
                    stt_imm(dde[:].bitcast(U32), dp[:].bitcast(U32),
                            0xFFFFF800, iot[:])

                    cand = selp.tile([128, 128], F32, name="cand")
                    for s in range(16):
                        nc.vector.max(out=cand[:, ts(s, 8)],
                                      in_=dde[:, ts(s, 128)])
                    m1 = selp.tile([128, 8], F32, name="m1")
                    m2 = selp.tile([128, 8], F32, name="m2")
                    m3 = selp.tile([128, 8], F32, name="m3")
                    cw = selp.tile([128, 128], F32, name="cw")
                    cw2 = selp.tile([128, 128], F32, name="cw2")
                    nc.vector.max(out=m1[:], in_=cand[:])
                    nc.vector.match_replace(out=cw[:], in_to_replace=m1[:],
                                            in_values=cand[:], imm_value=-1e30)
                    nc.vector.max(out=m2[:], in_=cw[:])
                    nc.vector.match_replace(out=cw2[:], in_to_replace=m2[:],
                                            in_values=cw[:], imm_value=-1e30)
                    nc.vector.max(out=m3[:], in_=cw2[:])

                    jgu = selp.tile([128, 24], U32, name="jgu")
                    ts_imm(jgu[:, 0:8], m1[:].bitcast(U32), 0x7FF)
                    ts_imm(jgu[:, 8:16], m2[:].bitcast(U32), 0x7FF)
                    ts_imm(jgu[:, 16:24], m3[:].bitcast(U32), 0x7FF)
                    jg = selp.tile([128, 20], F16, name="jg")
                    nc.vector.tensor_copy(jg[:], jgu[:, 0:20])

                    wp = wps.tile([128, 8, 20], F32, name="wp")
                    for c in range(8):
                        nc.tensor.matmul(wp[:, c, :], prm[:, c, :], jg[:],
                                         start=True, stop=True)
                    wrapped = selp.tile([128, 160], I16, name="wrapped")
                    nc.vector.tensor_copy(
                        wrapped[:].rearrange("q (s c) -> q s c", c=8),
                        wp[:].rearrange("q c s -> q s c"))

                    gb = gp.tile([128, 20, D], F32, name="gb")
                    q0 = (3 * t) % 4
                    nc.gpsimd.dma_gather(gb[:, 0:8, :], vslice, wrapped[:, 0:64],
                                         1024, 1024, D, queue_num=q0)
                    nc.gpsimd.dma_gather(gb[:, 8:16, :], vslice, wrapped[:, 64:128],
                                         1024, 1024, D, queue_num=(q0 + 1) % 4)
                    nc.gpsimd.dma_gather(gb[:, 16:20, :], vslice, wrapped[:, 128:160],
                                         512, 512, D, queue_num=(q0 + 2) % 4)

                    vm = gp.tile([128, D], F32, name="vm")
                    nc.vector.tensor_reduce(out=vm[:],
                                            in_=gb.rearrange("p m d -> p d m"),
                                            axis=AX.X, op=ALU.max)
                    nc.vector.tensor_tensor(out=h[:, t, :], in0=u[:, t, :],
                                            in1=vm[:], op=ALU.add)
        return h

    def transpose_prep(tc, layer, h, hA_next, hX_next, nsq_row, C2, idn_sb,
                       onescol_sb):
        """Build next layer's [feat; nsq; ones] (f32r) from h [128, NT, C2]."""
        with tc.tile_pool(name=f"L{layer}t", bufs=3, space="PSUM") as tps, \
             tc.tile_pool(name=f"L{layer}tsb", bufs=1) as tsbp:
            for t in range(NT):
                tp = tps.tile([C2, 128], F32, name="tp")
                nc.tensor.transpose(tp[:], h[:, t, 0:C2], idn_sb[:])
                nc.scalar.activation(hA_next[0:C2, ts(t, 128)], tp[:], AF.Copy,
                                     scale=1.0)
                nc.scalar.activation(hX_next[0:C2, ts(t, 128)], tp[:], AF.Copy,
                                     scale=2.0)
            xsq = tsbp.tile([C2, N], F32R, name="xsq")
            nc.scalar.square(xsq[:], hA_next[0:C2, :])
            for j in range(JC):
                sqp = tps.tile([1, 512], F32, name="sqp")
                nc.tensor.matmul(sqp[:], onescol_sb[0:C2, :], xsq[:, ts(j, 512)],
                                 start=True, stop=True)
                nc.scalar.activation(nsq_row[0:1, ts(j, 512)], sqp[:], AF.Copy,
                                     scale=-1.0)

    with TileContext(nc) as tc:
        with tc.tile_pool(name="const", bufs=1) as cp, \
             tc.tile_pool(name="feat", bufs=1) as fp, \
             tc.tile_pool(name="vdram", bufs=1, space="DRAM") as vdp:
            for _D in (64, 128, 256):
                v_drams[_D] = vdp.tile([N, _D], F32, name=f"v_dram{_D}")
            idn_sb = cp.tile([128, 128], F32)
            nc.sync.dma_start(idn_sb[:], idn)
            idnN_sb = cp.tile([128, 128], F32R)
            nc.sync.dma_start(idnN_sb[:], idnN)
            idnP_sb = cp.tile([128, 128], F32R)
            nc.sync.dma_start(idnP_sb[:], idnP)
            onesSB = cp.tile([1, N], F32R)
            nc.sync.dma_start(onesSB[:], onesr)
            onescol_sb = cp.tile([128, 1], F32R)
            nc.sync.dma_start(onescol_sb[:], onescol)
            iot = cp.tile([128, N], U32)
            nc.sync.dma_start(iot[:], iot_in)
            prm = cp.tile([128, 8, 128], F16)
            nc.sync.dma_start(prm[:], prm_in)
            ones1 = onesSB[0:1, 0:128]

            hA1_sb = fp.tile([5, N], F32R)
            nc.sync.dma_start(hA1_sb[:], hA1)
            hX1_sb = fp.tile([4, N], F32R)
            nc.sync.dma_start(hX1_sb[:], hX1)
            hA2_sb = fp.tile([66, N], F32R)
            hX2_sb = fp.tile([65, N], F32R)
            hA3_sb = fp.tile([128, N], F32R)
            hX3_sb = fp.tile([128, N], F32R)
            nsq3 = fp.tile([1, N], F32R)
            h3 = fp.tile([128, NT, 256], F32)

            with tc.tile_pool(name="wts", bufs=1) as wp_:
                w = {}
                for nm, ap_, shape in [("ABc1", ABc1, [5, 64]),
                                       ("BB1", BB1, [3, 64]),
                                       ("ABc2", ABc2, [66, 128]),
                                       ("BB2", BB2, [64, 128]),
                                       ("AB3", AB3, [128, 256]),
                                       ("BB3", BB3, [128, 256]),
                                       ("b3r", b3r, [1, 256])]:
                    t_ = wp_.tile(shape, F32R, name=f"w_{nm}")
                    nc.sync.dma_start(t_[:], ap_)
                    w[nm] = t_
                b3r_sb = w["b3r"]

                # ones rows for the device-built augmented feature tiles
                nc.sync.dma_start(hA2_sb[65:66, :], onesr)
                nc.sync.dma_start(hX2_sb[64:65, :], onesr)

                h1 = run_layer(tc, 1, 3, 64, hA1_sb, hX1_sb, None, ones1,
                               iot, prm, (w["ABc1"], w["BB1"]), None)
                transpose_prep(tc, 1, h1, hA2_sb, hX2_sb, hA2_sb[64:65, :],
                               64, idn_sb, onescol_sb)
                h2 = run_layer(tc, 2, 64, 128, hA2_sb, hX2_sb, None, ones1,
                               iot, prm, (w["ABc2"], w["BB2"]), None)
                transpose_prep(tc, 2, h2, hA3_sb, hX3_sb, nsq3,
                               128, idn_sb, onescol_sb)
                run_layer(tc, 3, 128, 256, hA3_sb, hX3_sb, nsq3, ones1,
                          iot, prm, (w["AB3"], w["BB3"]), h3)

            # ---------- global max pool + FC head ----------
            with tc.tile_pool(name="head", bufs=1) as hp, \
                 tc.tile_pool(name="headps", bufs=1, space="PSUM") as hps:
                gmax = hp.tile([128, 256], F32)
                nc.vector.tensor_reduce(out=gmax[:],
                                        in_=h3.rearrange("p g d -> p d g"),
                                        axis=AX.X, op=ALU.max)
                g0 = hp.tile([128, 1], F32)
                g1 = hp.tile([128, 1], F32)
                for half, gdst in ((0, g0), (1, g1)):
                    tp = hps.tile([128, 128], F32, name="tp", tag="tp")
                    nc.tensor.transpose(tp[:], gmax[:, ts(half, 128)], idn_sb[:])
                    tsb = hp.tile([128, 128], F32, name=f"tsb_{half}")
                    nc.scalar.copy(tsb[:], tp[:])
                    nc.vector.tensor_reduce(out=gdst[:], in_=tsb[:], axis=AX.X,
                                            op=ALU.max)

                fw1 = [hp.tile([128, 512], F32, name=f"fw1_{kk}") for kk in range(2)]
                fw2 = [hp.tile([128, 256], F32, name=f"fw2_{kk}") for kk in range(4)]
                fw3 = [hp.tile([128, 16], F32, name=f"fw3_{kk}") for kk in range(2)]
                fb1 = hp.tile([128, 4], F32)
                fb2 = hp.tile([128, 2], F32)
                fb3 = hp.tile([16, 1], F32)
                for kk in range(2):
                    nc.sync.dma_start(fw1[kk][:], fc1w[ts(kk, 128), :])
                    nc.sync.dma_start(fw3[kk][:], fc3w[ts(kk, 128), :])
                for kk in range(4):
                    nc.sync.dma_start(fw2[kk][:], fc2w[ts(kk, 128), :])
                nc.sync.dma_start(fb1[:], fc1b)
                nc.sync.dma_start(fb2[:], fc2b)
                nc.sync.dma_start(fb3[:], fc3b)

                a1 = [hp.tile([128, 1], F32, name=f"a1_{m}") for m in range(4)]
                for m in range(4):
                    p = hps.tile([128, 1], F32, name="fcp", tag="fcp")
                    nc.tensor.matmul(p[:], fw1[0][:, ts(m, 128)], g0[:],
                                     start=True, stop=False)
                    nc.tensor.matmul(p[:], fw1[1][:, ts(m, 128)], g1[:],
                                     start=False, stop=True)
                    nc.scalar.activation(a1[m][:], p[:], AF.Relu,
                                         bias=fb1[:, m:m + 1], scale=1.0)
                a2 = [hp.tile([128, 1], F32, name=f"a2_{m}") for m in range(2)]
                for m in range(2):
                    p = hps.tile([128, 1], F32, name="fcp", tag="fcp")
                    for kk in range(4):
                        nc.tensor.matmul(p[:], fw2[kk][:, ts(m, 128)], a1[kk][:],
                                         start=(kk == 0), stop=(kk == 3))
                    nc.scalar.activation(a2[m][:], p[:], AF.Relu,
                                         bias=fb2[:, m:m + 1], scale=1.0)
                p3 = hps.tile([128, 1], F32, name="fcp", tag="fcp")[0:16, :]
                for kk in range(2):
                    nc.tensor.matmul(p3[:], fw3[kk][:], a2[kk][:],
                                     start=(kk == 0), stop=(kk == 1))
                o_sb = hp.tile([16, 1], F32)
                nc.scalar.activation(o_sb[:], p3[:], AF.Identity, bias=fb3[:],
                                     scale=1.0)
                nc.sync.dma_start(out, o_sb[:])

    nc.finalize()
    return nc


def get_nc():
    if 0 not in _NC_CACHE:
        _NC_CACHE[0] = _builder()
    return _NC_CACHE[0]


def make_in_maps(x, W1, b1, W2, b2, W3, b3, fc1_w, fc1_b, fc2_w, fc2_b,
                 fc3_w, fc3_b):
    f32 = np.float32
    x = np.asarray(x, f32)
    B = x.shape[0]
    W1, W2, W3 = np.asarray(W1, f32), np.asarray(W2, f32), np.asarray(W3, f32)
    prm = np.zeros((128, 8, 128), dtype=np.float16)
    for c in range(8):
        for j in range(128):
            prm[16 * c + (j % 16), c, j] = 1.0
    shared = {
        "ABc1": np.concatenate([W1[:3] - W1[3:6], np.zeros((1, 64), f32),
                                np.asarray(b1, f32)[None]], 0),
        "BB1": np.ascontiguousarray(W1[3:6]),
        "ABc2": np.concatenate([W2[:64] - W2[64:], np.zeros((1, 128), f32),
                                np.asarray(b2, f32)[None]], 0),
        "BB2": np.ascontiguousarray(W2[64:]),
        "AB3": np.ascontiguousarray(W3[:128] - W3[128:]),
        "BB3": np.ascontiguousarray(W3[128:]),
        "b3r": np.asarray(b3, f32)[None],
        "idn": np.eye(128, dtype=f32),
        "idnN": (np.eye(128) * -1e30).astype(f32),
        "idnP": np.eye(128, dtype=f32),
        "onesr": np.ones((1, N), f32),
        "onescol": np.ones((128, 1), f32),
        "iot": np.broadcast_to(np.arange(N, dtype=np.uint32),
                               (128, N)).copy(),
        "prm": prm,
        "fc1w": np.asarray(fc1_w, f32),
        "fc1b": np.ascontiguousarray(np.asarray(fc1_b, f32).reshape(4, 128).T),
        "fc2w": np.asarray(fc2_w, f32),
        "fc2b": np.ascontiguousarray(np.asarray(fc2_b, f32).reshape(2, 128).T),
        "fc3w": np.pad(np.asarray(fc3_w, f32), ((0, 0), (0, 6))),
        "fc3b": np.pad(np.asarray(fc3_b, f32), (0, 6))[:, None],
    }
    in_maps = []
    for bb in range(B):
        xb = x[bb]
        xT = np.ascontiguousarray(xb.T)
        nsq = -(xb * xb).sum(-1)[None, :].astype(f32)
        m = dict(shared)
        m["hA1"] = np.concatenate([xT, nsq, np.ones((1, N), f32)], 0)
        m["hX1"] = np.concatenate([2.0 * xT, np.ones((1, N), f32)], 0)
        in_maps.append(m)
    return in_maps


def kernel(x, k, W1, b1, W2, b2, W3, b3, fc1_w, fc1_b, fc2_w, fc2_b, fc3_w,
           fc3_b):
    from concourse import bass_utils
    x = np.asarray(x)
    assert int(k) == 20 and x.shape[1] == N and x.shape[2] == 3
    B = x.shape[0]
    assert B == 8
    nc = get_nc()
    in_maps = make_in_maps(x, W1, b1, W2, b2, W3, b3,
                           fc1_w, fc1_b, fc2_w, fc2_b, fc3_w, fc3_b)
    res = bass_utils.run_bass_kernel_spmd(nc, in_maps, core_ids=list(range(B)))
    outs = np.stack([res.results[bb]["out"][:10, 0] for bb in range(B)], axis=0)
    return outs.astype(np.float32)


# revision 7
# speedup vs baseline: 1.0172x; 1.0172x over previous
"""DGCNN (3x DynamicEdgeConv + global max pool + MLP head) on 8 Trainium2
NeuronCores, data-parallel over the batch (one point cloud per core).

EdgeConv algebra: h_ij = [x_i, x_j - x_i] @ W + b = u_i + v_j with
  u = x @ (Wa - Wb) + b,  v = x @ Wb;  out_i = u_i + max_{j in knn(i)} v_j.

kNN key d''_ij = 2 x_i.x_j - |x_j|^2 (largest = nearest); the self column is
killed with a -1e30 diagonal matmul so the top-20 are exactly the neighbors.
Distances are computed with fp32r matmuls whose contraction is augmented with
a ones/nsq row pair (one matmul per 512-col chunk). The column index is
embedded into the low 11 mantissa bits of each distance (bitwise AND+OR with
an iota row), so top-k selection needs no max_index passes: 16 segment max8's
produce 128 candidates, 3x max8 + 2x match_replace pick the top-24, and the
indices pop out of the winning values with a bitwise AND.
Neighbor v-rows are fetched with 3 batched dma_gather calls (1024+1024+512
rows) whose int16 index list is built by 8 permutation matmuls that transpose
jtab into the gather's wrapped 16-partition layout. The 20-way neighbor max
is a single DVE tensor_reduce, added to u to form the layer output.
"""
import numpy as np

_NC_CACHE = {}

N, NT, JC = 2048, 16, 4


def _builder():
    import concourse.bacc as bacc
    import concourse.mybir as mybir
    from concourse.tile import TileContext

    F32 = mybir.dt.float32
    F32R = mybir.dt.float32r
    F16 = mybir.dt.float16
    U32 = mybir.dt.uint32
    I16 = mybir.dt.int16
    AF = mybir.ActivationFunctionType
    ALU = mybir.AluOpType
    AX = mybir.AxisListType

    def ts(i, s):
        return slice(i * s, (i + 1) * s)

    nc = bacc.Bacc("TRN2", num_devices=8, num_swdge_queues=4)

    def din(name, shape, dt=F32R):
        return nc.dram_tensor(name, shape, dt, kind="ExternalInput").ap()

    hA1 = din("hA1", [5, N])            # [x^T; nsq; ones]
    hX1 = din("hX1", [4, N])            # [2 x^T; ones]
    ABc1 = din("ABc1", [5, 64])
    BB1 = din("BB1", [3, 64])
    ABc2 = din("ABc2", [66, 128])
    BB2 = din("BB2", [64, 128])
    AB3 = din("AB3", [128, 256])
    BB3 = din("BB3", [128, 256])
    b3r = din("b3r", [1, 256])
    idn = din("idn", [128, 128], F32)
    idnN = din("idnN", [128, 128])      # -1e30 * I
    idnP = din("idnP", [128, 128])      # I
    onesr = din("onesr", [1, N])        # ones row
    onescol = din("onescol", [128, 1])  # ones column
    iot_in = din("iot", [128, N], U32)
    prm_in = din("prm", [128, 8, 128], F16)
    fc1w = din("fc1w", [256, 512], F32)
    fc1b = din("fc1b", [128, 4], F32)
    fc2w = din("fc2w", [512, 256], F32)
    fc2b = din("fc2b", [128, 2], F32)
    fc3w = din("fc3w", [256, 16], F32)
    fc3b = din("fc3b", [16, 1], F32)
    out = nc.dram_tensor("out", [16, 1], F32, kind="ExternalOutput").ap()

    def ts_imm(out_ap, in0, imm):
        eng = nc.vector
        return eng.add_instruction(
            mybir.InstTensorScalarPtr(
                name=eng.bass.get_next_instruction_name(),
                op0=ALU.bitwise_and, op1=ALU.bypass,
                ins=[eng.lower_ap(in0),
                     mybir.ImmediateValue(dtype=U32, value=imm)],
                outs=[eng.lower_ap(out_ap)]))

    def stt_imm(out_ap, in0, imm, in1):
        eng = nc.vector
        return eng.add_instruction(
            mybir.InstTensorScalarPtr(
                name=eng.bass.get_next_instruction_name(),
                is_scalar_tensor_tensor=True,
                op0=ALU.bitwise_and, op1=ALU.bitwise_or,
                ins=[eng.lower_ap(in0),
                     mybir.ImmediateValue(dtype=U32, value=imm),
                     eng.lower_ap(in1)],
                outs=[eng.lower_ap(out_ap)]))

    v_drams = {}

    def run_layer(tc, layer, C, D, hA, hX, nsq_l3, ones1, iot, prm, u_cfg, out_h):
        """hA: [C+2, N] (feat, nsq, ones) f32r for L1/L2; [128, N] feat for L3.
        hX: [C+1, N] (2feat, ones) for L1/L2; [128, N] 2feat for L3.
        Returns h [128, NT, D] f32 (i = t*128 + p)."""
        l3 = layer == 3
        vslice = v_drams[D]
        ABc, BB = u_cfg
        with tc.tile_pool(name=f"L{layer}", bufs=1) as lp:
            u = lp.tile([128, NT, D], F32, name=f"u_{layer}")
            # ---------- u/v matmuls, v -> DRAM ----------
            with tc.tile_pool(name=f"L{layer}uv", bufs=4, space="PSUM") as uvps, \
                 tc.tile_pool(name=f"L{layer}uvsb", bufs=4) as uvsb:
                for t in range(NT):
                    vp = uvps.tile([128, D], F32, name="vp")
                    nc.tensor.matmul(vp[:], hA[0:C, ts(t, 128)], BB[:],
                                     start=True, stop=True)
                    up = uvps.tile([128, D], F32, name="up")
                    if l3:
                        nc.tensor.matmul(up[:], hA[:, ts(t, 128)], ABc[:],
                                         start=True, stop=False)
                        nc.tensor.matmul(up[:], ones1, b3r_sb[:],
                                         start=False, stop=True)
                    else:
                        nc.tensor.matmul(up[:], hA[0:C + 2, ts(t, 128)], ABc[:],
                                         start=True, stop=True)
                    vsb = uvsb.tile([128, D], F32, name="vsb")
                    nc.scalar.copy(vsb[:], vp[:])
                    nc.scalar.copy(u[:, t, :], up[:])
                    nc.sync.dma_start(vslice[ts(t, 128), :], vsb[:])

            # ---------- per-tile: dist + select + gather + reduce ----------
            h = out_h if out_h is not None else lp.tile([128, NT, D], F32,
                                                        name=f"h_{layer}")
            with tc.tile_pool(name=f"L{layer}d", bufs=1, space="PSUM") as dps, \
                 tc.tile_pool(name=f"L{layer}w", bufs=2, space="PSUM") as wps, \
                 tc.tile_pool(name=f"L{layer}dd", bufs=2) as ddp, \
                 tc.tile_pool(name=f"L{layer}sel", bufs=2) as selp, \
                 tc.tile_pool(name=f"L{layer}g", bufs=3) as gp:
                pending = []

                def flush_pending():
                    gb_p, t_p = pending.pop(0)
                    vm = gp.tile([128, D], F32, name="vm")
                    nc.vector.tensor_reduce(out=vm[:],
                                            in_=gb_p.rearrange("p m d -> p d m"),
                                            axis=AX.X, op=ALU.max)
                    nc.vector.tensor_tensor(out=h[:, t_p, :], in0=u[:, t_p, :],
                                            in1=vm[:], op=ALU.add)

                for t in range(NT):
                    dp = dps.tile([128, N], F32, name="dp")
                    tchunk = t // 4
                    for j in range(JC):
                        last = (j != tchunk)
                        if l3:
                            nc.tensor.matmul(dp[:, ts(j, 512)], hX[:, ts(t, 128)],
                                             hA[:, ts(j, 512)],
                                             start=True, stop=False)
                            nc.tensor.matmul(dp[:, ts(j, 512)], ones1,
                                             nsq_l3[:, ts(j, 512)],
                                             start=False, stop=last)
                        else:
                            nc.tensor.matmul(dp[:, ts(j, 512)], hX[:, ts(t, 128)],
                                             hA[0:C + 1, ts(j, 512)],
                                             start=True, stop=last)
                        if not last:
                            nc.tensor.matmul(dp[:, ts(t, 128)], idnN_sb[:],
                                             idnP_sb[:], start=False, stop=True,
                                             skip_group_check=True)
                    dde = ddp.tile([128, N], F32, name="dde")
                    stt_imm(dde[:].bitcast(U32), dp[:].bitcast(U32),
                            0xFFFFF800, iot[:])

                    cand = selp.tile([128, 128], F32, name="cand")
                    for s in range(16):
                        nc.vector.max(out=cand[:, ts(s, 8)],
                                      in_=dde[:, ts(s, 128)])
                    m1 = selp.tile([128, 8], F32, name="m1")
                    m2 = selp.tile([128, 8], F32, name="m2")
                    m3 = selp.tile([128, 8], F32, name="m3")
                    cw = selp.tile([128, 128], F32, name="cw")
                    cw2 = selp.tile([128, 128], F32, name="cw2")
                    nc.vector.max(out=m1[:], in_=cand[:])
                    nc.vector.match_replace(out=cw[:], in_to_replace=m1[:],
                                            in_values=cand[:], imm_value=-1e30)
                    nc.vector.max(out=m2[:], in_=cw[:])
                    nc.vector.match_replace(out=cw2[:], in_to_replace=m2[:],
                                            in_values=cw[:], imm_value=-1e30)
                    nc.vector.max(out=m3[:], in_=cw2[:])

                    jgu = selp.tile([128, 24], U32, name="jgu")
                    ts_imm(jgu[:, 0:8], m1[:].bitcast(U32), 0x7FF)
                    ts_imm(jgu[:, 8:16], m2[:].bitcast(U32), 0x7FF)
                    ts_imm(jgu[:, 16:24], m3[:].bitcast(U32), 0x7FF)
                    jg = selp.tile([128, 20], F16, name="jg")
                    nc.vector.tensor_copy(jg[:], jgu[:, 0:20])

                    wp = wps.tile([128, 8, 20], F32, name="wp")
                    for c in range(8):
                        nc.tensor.matmul(wp[:, c, :], prm[:, c, :], jg[:],
                                         start=True, stop=True)
                    wrapped = selp.tile([128, 160], I16, name="wrapped")
                    nc.vector.tensor_copy(
                        wrapped[:].rearrange("q (s c) -> q s c", c=8),
                        wp[:].rearrange("q c s -> q s c"))

                    gb = gp.tile([128, 20, D], F32, name="gb")
                    q0 = (3 * t) % 4
                    nc.gpsimd.dma_gather(gb[:, 0:8, :], vslice, wrapped[:, 0:64],
                                         1024, 1024, D, queue_num=q0)
                    nc.gpsimd.dma_gather(gb[:, 8:16, :], vslice, wrapped[:, 64:128],
                                         1024, 1024, D, queue_num=(q0 + 1) % 4)
                    nc.gpsimd.dma_gather(gb[:, 16:20, :], vslice, wrapped[:, 128:160],
                                         512, 512, D, queue_num=(q0 + 2) % 4)
                    pending.append((gb, t))
                    if len(pending) > 1:
                        flush_pending()
                while pending:
                    flush_pending()
        return h

    def transpose_prep(tc, layer, h, hA_next, hX_next, nsq_row, C2, idn_sb,
                       onescol_sb):
        """Build next layer's [feat; nsq; ones] (f32r) from h [128, NT, C2]."""
        with tc.tile_pool(name=f"L{layer}t", bufs=3, space="PSUM") as tps, \
             tc.tile_pool(name=f"L{layer}tsb", bufs=1) as tsbp:
            for t in range(NT):
                tp = tps.tile([C2, 128], F32, name="tp")
                nc.tensor.transpose(tp[:], h[:, t, 0:C2], idn_sb[:])
                nc.scalar.activation(hA_next[0:C2, ts(t, 128)], tp[:], AF.Copy,
                                     scale=1.0)
                nc.scalar.activation(hX_next[0:C2, ts(t, 128)], tp[:], AF.Copy,
                                     scale=2.0)
            xsq = tsbp.tile([C2, N], F32R, name="xsq")
            nc.scalar.square(xsq[:], hA_next[0:C2, :])
            for j in range(JC):
                sqp = tps.tile([1, 512], F32, name="sqp")
                nc.tensor.matmul(sqp[:], onescol_sb[0:C2, :], xsq[:, ts(j, 512)],
                                 start=True, stop=True)
                nc.scalar.activation(nsq_row[0:1, ts(j, 512)], sqp[:], AF.Copy,
                                     scale=-1.0)

    with TileContext(nc) as tc:
        with tc.tile_pool(name="const", bufs=1) as cp, \
             tc.tile_pool(name="feat", bufs=1) as fp, \
             tc.tile_pool(name="vdram", bufs=1, space="DRAM") as vdp:
            for _D in (64, 128, 256):
                v_drams[_D] = vdp.tile([N, _D], F32, name=f"v_dram{_D}")
            idn_sb = cp.tile([128, 128], F32)
            nc.sync.dma_start(idn_sb[:], idn)
            idnN_sb = cp.tile([128, 128], F32R)
            nc.sync.dma_start(idnN_sb[:], idnN)
            idnP_sb = cp.tile([128, 128], F32R)
            nc.sync.dma_start(idnP_sb[:], idnP)
            onesSB = cp.tile([1, N], F32R)
            nc.sync.dma_start(onesSB[:], onesr)
            onescol_sb = cp.tile([128, 1], F32R)
            nc.sync.dma_start(onescol_sb[:], onescol)
            iot = cp.tile([128, N], U32)
            nc.sync.dma_start(iot[:], iot_in)
            prm = cp.tile([128, 8, 128], F16)
            nc.sync.dma_start(prm[:], prm_in)
            ones1 = onesSB[0:1, 0:128]

            hA1_sb = fp.tile([5, N], F32R)
            nc.sync.dma_start(hA1_sb[:], hA1)
            hX1_sb = fp.tile([4, N], F32R)
            nc.sync.dma_start(hX1_sb[:], hX1)
            hA2_sb = fp.tile([66, N], F32R)
            hX2_sb = fp.tile([65, N], F32R)
            hA3_sb = fp.tile([128, N], F32R)
            hX3_sb = fp.tile([128, N], F32R)
            nsq3 = fp.tile([1, N], F32R)
            h3 = fp.tile([128, NT, 256], F32)

            with tc.tile_pool(name="wts", bufs=1) as wp_:
                w = {}
                for nm, ap_, shape in [("ABc1", ABc1, [5, 64]),
                                       ("BB1", BB1, [3, 64]),
                                       ("ABc2", ABc2, [66, 128]),
                                       ("BB2", BB2, [64, 128]),
                                       ("AB3", AB3, [128, 256]),
                                       ("BB3", BB3, [128, 256]),
                                       ("b3r", b3r, [1, 256])]:
                    t_ = wp_.tile(shape, F32R, name=f"w_{nm}")
                    nc.sync.dma_start(t_[:], ap_)
                    w[nm] = t_
                b3r_sb = w["b3r"]

                # ones rows for the device-built augmented feature tiles
                nc.sync.dma_start(hA2_sb[65:66, :], onesr)
                nc.sync.dma_start(hX2_sb[64:65, :], onesr)

                h1 = run_layer(tc, 1, 3, 64, hA1_sb, hX1_sb, None, ones1,
                               iot, prm, (w["ABc1"], w["BB1"]), None)
                transpose_prep(tc, 1, h1, hA2_sb, hX2_sb, hA2_sb[64:65, :],
                               64, idn_sb, onescol_sb)
                h2 = run_layer(tc, 2, 64, 128, hA2_sb, hX2_sb, None, ones1,
                               iot, prm, (w["ABc2"], w["BB2"]), None)
                transpose_prep(tc, 2, h2, hA3_sb, hX3_sb, nsq3,
                               128, idn_sb, onescol_sb)
                run_layer(tc, 3, 128, 256, hA3_sb, hX3_sb, nsq3, ones1,
                          iot, prm, (w["AB3"], w["BB3"]), h3)

            # ---------- global max pool + FC head ----------
            with tc.tile_pool(name="head", bufs=1) as hp, \
                 tc.tile_pool(name="headps", bufs=1, space="PSUM") as hps:
                gmax = hp.tile([128, 256], F32)
                nc.vector.tensor_reduce(out=gmax[:],
                                        in_=h3.rearrange("p g d -> p d g"),
                                        axis=AX.X, op=ALU.max)
                g0 = hp.tile([128, 1], F32)
                g1 = hp.tile([128, 1], F32)
                for half, gdst in ((0, g0), (1, g1)):
                    tp = hps.tile([128, 128], F32, name="tp", tag="tp")
                    nc.tensor.transpose(tp[:], gmax[:, ts(half, 128)], idn_sb[:])
                    tsb = hp.tile([128, 128], F32, name=f"tsb_{half}")
                    nc.scalar.copy(tsb[:], tp[:])
                    nc.vector.tensor_reduce(out=gdst[:], in_=tsb[:], axis=AX.X,
                                            op=ALU.max)

                fw1 = [hp.tile([128, 512], F32, name=f"fw1_{kk}") for kk in range(2)]
                fw2 = [hp.tile([128, 256], F32, name=f"fw2_{kk}") for kk in range(4)]
                fw3 = [hp.tile([128, 16], F32, name=f"fw3_{kk}") for kk in range(2)]
                fb1 = hp.tile([128, 4], F32)
                fb2 = hp.tile([128, 2], F32)
                fb3 = hp.tile([16, 1], F32)
                for kk in range(2):
                    nc.sync.dma_start(fw1[kk][:], fc1w[ts(kk, 128), :])
                    nc.sync.dma_start(fw3[kk][:], fc3w[ts(kk, 128), :])
                for kk in range(4):
                    nc.sync.dma_start(fw2[kk][:], fc2w[ts(kk, 128), :])
                nc.sync.dma_start(fb1[:], fc1b)
                nc.sync.dma_start(fb2[:], fc2b)
                nc.sync.dma_start(fb3[:], fc3b)

                a1 = [hp.tile([128, 1], F32, name=f"a1_{m}") for m in range(4)]
                for m in range(4):
                    p = hps.tile([128, 1], F32, name="fcp", tag="fcp")
                    nc.tensor.matmul(p[:], fw1[0][:, ts(m, 128)], g0[:],
                                     start=True, stop=False)
                    nc.tensor.matmul(p[:], fw1[1][:, ts(m, 128)], g1[:],
                                     start=False, stop=True)
                    nc.scalar.activation(a1[m][:], p[:], AF.Relu,
                                         bias=fb1[:, m:m + 1], scale=1.0)
                a2 = [hp.tile([128, 1], F32, name=f"a2_{m}") for m in range(2)]
                for m in range(2):
                    p = hps.tile([128, 1], F32, name="fcp", tag="fcp")
                    for kk in range(4):
                        nc.tensor.matmul(p[:], fw2[kk][:, ts(m, 128)], a1[kk][:],
                                         start=(kk == 0), stop=(kk == 3))
                    nc.scalar.activation(a2[m][:], p[:], AF.Relu,
                                         bias=fb2[:, m:m + 1], scale=1.0)
                p3 = hps.tile([128, 1], F32, name="fcp", tag="fcp")[0:16, :]
                for kk in range(2):
                    nc.tensor.matmul(p3[:], fw3[kk][:], a2[kk][:],
                                     start=(kk == 0), stop=(kk == 1))
                o_sb = hp.tile([16, 1], F32)
                nc.scalar.activation(o_sb[:], p3[:], AF.Identity, bias=fb3[:],
                                     scale=1.0)
                nc.sync.dma_start(out, o_sb[:])

    nc.finalize()
    return nc


def get_nc():
    if 0 not in _NC_CACHE:
        _NC_CACHE[0] = _builder()
    return _NC_CACHE[0]


def make_in_maps(x, W1, b1, W2, b2, W3, b3, fc1_w, fc1_b, fc2_w, fc2_b,
                 fc3_w, fc3_b):
    f32 = np.float32
    x = np.asarray(x, f32)
    B = x.shape[0]
    W1, W2, W3 = np.asarray(W1, f32), np.asarray(W2, f32), np.asarray(W3, f32)
    prm = np.zeros((128, 8, 128), dtype=np.float16)
    for c in range(8):
        for j in range(128):
            prm[16 * c + (j % 16), c, j] = 1.0
    shared = {
        "ABc1": np.concatenate([W1[:3] - W1[3:6], np.zeros((1, 64), f32),
                                np.asarray(b1, f32)[None]], 0),
        "BB1": np.ascontiguousarray(W1[3:6]),
        "ABc2": np.concatenate([W2[:64] - W2[64:], np.zeros((1, 128), f32),
                                np.asarray(b2, f32)[None]], 0),
        "BB2": np.ascontiguousarray(W2[64:]),
        "AB3": np.ascontiguousarray(W3[:128] - W3[128:]),
        "BB3": np.ascontiguousarray(W3[128:]),
        "b3r": np.asarray(b3, f32)[None],
        "idn": np.eye(128, dtype=f32),
        "idnN": (np.eye(128) * -1e30).astype(f32),
        "idnP": np.eye(128, dtype=f32),
        "onesr": np.ones((1, N), f32),
        "onescol": np.ones((128, 1), f32),
        "iot": np.broadcast_to(np.arange(N, dtype=np.uint32),
                               (128, N)).copy(),
        "prm": prm,
        "fc1w": np.asarray(fc1_w, f32),
        "fc1b": np.ascontiguousarray(np.asarray(fc1_b, f32).reshape(4, 128).T),
        "fc2w": np.asarray(fc2_w, f32),
        "fc2b": np.ascontiguousarray(np.asarray(fc2_b, f32).reshape(2, 128).T),
        "fc3w": np.pad(np.asarray(fc3_w, f32), ((0, 0), (0, 6))),
        "fc3b": np.pad(np.asarray(fc3_b, f32), (0, 6))[:, None],
    }
    in_maps = []
    for bb in range(B):
        xb = x[bb]
        xT = np.ascontiguousarray(xb.T)
        nsq = -(xb * xb).sum(-1)[None, :].astype(f32)
        m = dict(shared)
        m["hA1"] = np.concatenate([xT, nsq, np.ones((1, N), f32)], 0)
        m["hX1"] = np.concatenate([2.0 * xT, np.ones((1, N), f32)], 0)
        in_maps.append(m)
    return in_maps


def kernel(x, k, W1, b1, W2, b2, W3, b3, fc1_w, fc1_b, fc2_w, fc2_b, fc3_w,
           fc3_b):
    from concourse import bass_utils
    x = np.asarray(x)
    assert int(k) == 20 and x.shape[1] == N and x.shape[2] == 3
    B = x.shape[0]
    assert B == 8
    nc = get_nc()
    in_maps = make_in_maps(x, W1, b1, W2, b2, W3, b3,
                           fc1_w, fc1_b, fc2_w, fc2_b, fc3_w, fc3_b)
    res = bass_utils.run_bass_kernel_spmd(nc, in_maps, core_ids=list(range(B)))
    outs = np.stack([res.results[bb]["out"][:10, 0] for bb in range(B)], axis=0)
    return outs.astype(np.float32)


# revision 22
# speedup vs baseline: 1.3552x; 1.3323x over previous
"""DGCNN (3x DynamicEdgeConv + global max pool + MLP head) on 8 Trainium2
NeuronCores, data-parallel over the batch (one point cloud per core).

EdgeConv algebra: h_ij = [x_i, x_j - x_i] @ W + b = u_i + v_j with
  u = x @ (Wa - Wb) + b,  v = x @ Wb;  out_i = u_i + max_{j in knn(i)} v_j.

kNN key d''_ij = 2 x_i.x_j - |x_j|^2 (largest = nearest); the self column is
killed with a -1e30 diagonal matmul so the top-20 are exactly the neighbors.
Distances use fp32r matmuls; the contraction is augmented so one matmul per
512-col chunk computes 2x.x + nsq (layers 1/2).  Feature rows are stored as
hA = [nsq; feat; ones], hX = [ones; 2*feat] so u/v matmuls slice [feat; ones]
and never touch the late-computed nsq row, letting next-layer transposes and
u/v matmuls interleave into the current layer's selection loop.

The column index is embedded in the low 11 mantissa bits of each distance
(bitwise AND+OR with an iota row) so top-k selection needs no max_index: 16
segment max8's yield 128 candidates, 3x max8 + 2x match_replace pick the
top-24, and indices pop out of the winning values with a bitwise AND.
Neighbor v-rows are fetched with 3 batched dma_gather calls (1024+1024+512
rows) over 4 SWDGE queues; their int16 index list is built by 8 permutation
matmuls that transpose jtab into the gather's wrapped 16-partition layout.
The 20-way neighbor max is one DVE tensor_reduce (deferred 2 tiles to hide
gather latency), added to u to form the layer output.
"""
import numpy as np

_NC_CACHE = {}

N, NT, JC = 2048, 16, 4


def _builder():
    import concourse.bacc as bacc
    import concourse.mybir as mybir
    from concourse.tile import TileContext

    F32 = mybir.dt.float32
    F32R = mybir.dt.float32r
    F16 = mybir.dt.float16
    U32 = mybir.dt.uint32
    I16 = mybir.dt.int16
    AF = mybir.ActivationFunctionType
    ALU = mybir.AluOpType
    AX = mybir.AxisListType

    def ts(i, s):
        return slice(i * s, (i + 1) * s)

    nc = bacc.Bacc("TRN2", num_devices=8, num_swdge_queues=4)

    def din(name, shape, dt=F32R):
        return nc.dram_tensor(name, shape, dt, kind="ExternalInput").ap()

    hA1 = din("hA1", [4, N])            # [x^T; ones]
    nsq1 = din("nsq1", [1, N])          # -0.5 |x|^2
    ABc1 = din("ABc1", [4, 64])         # [W1a-W1b; b1]
    BB1 = din("BB1", [3, 64])
    ABc2 = din("ABc2", [65, 128])       # [W2a-W2b; b2]
    BB2 = din("BB2", [64, 128])
    AB3 = din("AB3", [128, 256])
    BB3 = din("BB3", [128, 256])
    b3r = din("b3r", [1, 256])
    idn = din("idn", [128, 128], F32)
    idnN = din("idnN", [128, 128])      # -1e30 * I
    idnP = din("idnP", [128, 128])      # I
    onesr = din("onesr", [1, N])        # ones row
    onescol = din("onescol", [128, 1])  # ones column
    iot_in = din("iot", [128, N], U32)
    prm_in = din("prm", [128, 8, 128], F16)
    fc1w = din("fc1w", [256, 512], F32)
    fc1b = din("fc1b", [128, 4], F32)
    fc2w = din("fc2w", [512, 256], F32)
    fc2b = din("fc2b", [128, 2], F32)
    fc3w = din("fc3w", [256, 16], F32)
    fc3b = din("fc3b", [16, 1], F32)
    out = nc.dram_tensor("out", [16, 1], F32, kind="ExternalOutput").ap()

    def ts_imm(out_ap, in0, imm):
        eng = nc.vector
        return eng.add_instruction(
            mybir.InstTensorScalarPtr(
                name=eng.bass.get_next_instruction_name(),
                op0=ALU.bitwise_and, op1=ALU.bypass,
                ins=[eng.lower_ap(in0),
                     mybir.ImmediateValue(dtype=U32, value=imm)],
                outs=[eng.lower_ap(out_ap)]))

    def stt_imm(out_ap, in0, imm, in1):
        eng = nc.vector
        return eng.add_instruction(
            mybir.InstTensorScalarPtr(
                name=eng.bass.get_next_instruction_name(),
                is_scalar_tensor_tensor=True,
                op0=ALU.bitwise_and, op1=ALU.bitwise_or,
                ins=[eng.lower_ap(in0),
                     mybir.ImmediateValue(dtype=U32, value=imm),
                     eng.lower_ap(in1)],
                outs=[eng.lower_ap(out_ap)]))

    from contextlib import ExitStack
    with TileContext(nc) as tc, ExitStack() as stack:
        cp = stack.enter_context(tc.tile_pool(name="const", bufs=1))
        fp = stack.enter_context(tc.tile_pool(name="feat", bufs=1))
        vdp = stack.enter_context(tc.tile_pool(name="vdram", bufs=1, space="DRAM"))
        uvps = stack.enter_context(tc.tile_pool(name="uvps", bufs=1, space="PSUM"))
        tps = stack.enter_context(tc.tile_pool(name="tps", bufs=1, space="PSUM"))
        uvsb = stack.enter_context(tc.tile_pool(name="uvsb", bufs=4))

        v_drams = {}
        for _D in (64, 128, 256):
            v_drams[_D] = vdp.tile([N, _D], F32, name=f"v_dram{_D}")
        idn_sb = cp.tile([128, 128], F32)
        nc.sync.dma_start(idn_sb[:], idn)
        idnN_sb = cp.tile([128, 128], F32R)
        nc.sync.dma_start(idnN_sb[:], idnN)
        idnP_sb = cp.tile([128, 128], F32R)
        nc.sync.dma_start(idnP_sb[:], idnP)
        onesSB = cp.tile([65, 128], F32R)
        nc.sync.dma_start(onesSB[0:1, :], onesr[0:1, 0:128])
        nc.sync.dma_start(onesSB[32:33, :], onesr[0:1, 0:128])
        nc.sync.dma_start(onesSB[64:65, :], onesr[0:1, 0:128])
        nsqall = cp.tile([65, N], F32R)
        onescol_sb = cp.tile([128, 1], F32R)
        nc.sync.dma_start(onescol_sb[:], onescol)
        iot = cp.tile([128, N], U32)
        nc.sync.dma_start(iot[:], iot_in)
        prm = cp.tile([128, 8, 128], F16)
        nc.sync.dma_start(prm[:], prm_in)
        ones1 = onesSB[0:1, 0:128]
        nsq1_sb = nsqall[0:1, :]
        nsq2_sb = nsqall[32:33, :]
        nsq3 = nsqall[64:65, :]
        nc.sync.dma_start(nsq1_sb, nsq1)

        w = {}
        for nm, ap_, shape in [("ABc1", ABc1, [4, 64]), ("BB1", BB1, [3, 64]),
                               ("ABc2", ABc2, [65, 128]), ("BB2", BB2, [64, 128]),
                               ("AB3", AB3, [128, 256]), ("BB3", BB3, [128, 256]),
                               ("b3r", b3r, [1, 256])]:
            t_ = cp.tile(shape, F32R, name=f"w_{nm}")
            nc.sync.dma_start(t_[:], ap_)
            w[nm] = t_

        def emit_uv(t, hA, C, D, ABc, BB, u, vslice, l3):
            """u/v matmuls for tile t; hA rows [nsq(0); feat(1..C); ones(C+1)]
            (L3: hA = feat only, bias via ones1 @ b3r)."""
            pair = uvps.tile([128, 512], F32, name="uvpair")
            up = pair[:, 0:D]
            vp = pair[:, 256:256 + D]
            if l3:
                nc.tensor.matmul(vp, hA[:, ts(t, 128)], BB[:],
                                 start=True, stop=True, skip_group_check=True)
                nc.tensor.matmul(up, hA[:, ts(t, 128)], ABc[:],
                                 start=True, stop=False, skip_group_check=True)
                nc.tensor.matmul(up, ones1, w["b3r"][:],
                                 start=False, stop=True, skip_group_check=True)
            else:
                nc.tensor.matmul(vp, hA[0:C, ts(t, 128)], BB[:],
                                 start=True, stop=True, skip_group_check=True)
                nc.tensor.matmul(up, hA[0:C + 1, ts(t, 128)], ABc[:],
                                 start=True, stop=True, skip_group_check=True)
            vsb = uvsb.tile([128, 256], F32, name="vsb")[:, 0:D]
            nc.scalar.copy(vsb, vp)
            nc.scalar.copy(u[:, t, :], up)
            nc.sync.dma_start(vslice[ts(t, 128), :], vsb)

        def emit_next_prep(t, h, nxt):
            """Transpose h tile t into next layer's hA/hX (+per-tile square)."""
            C2 = nxt["C"]
            tp = tps.tile([128, 128], F32, name="tp", tag="tp")[0:C2, :]
            nc.tensor.transpose(tp, h[:, t, 0:C2], idn_sb[:])
            nc.scalar.activation(nxt["hA"][0:C2, ts(t, 128)], tp, AF.Copy,
                                 scale=1.0)
            nc.scalar.square(nxt["xsq"][0:C2, ts(t, 128)], tp)
            emit_uv(t, nxt["hA"], C2, nxt["D"], nxt["ABc"], nxt["BB"],
                    nxt["u"], nxt["vD"], nxt["l3"])

        def finalize_nsq(nxt):
            C2 = nxt["C"]
            for j in range(JC):
                sqp = tps.tile([1, 512], F32, name="sqp", tag="sqp")
                nc.tensor.matmul(sqp[:], onescol_sb[0:C2, :],
                                 nxt["xsq"][0:C2, ts(j, 512)],
                                 start=True, stop=True)
                nc.scalar.activation(nxt["nsqrow"][0:1, ts(j, 512)], sqp[:],
                                     AF.Copy, scale=-0.5)

        def run_layer(tc, layer, C, D, hA, nsq_t, ones_row, u, h, nxt):
            """Selection + gather + reduce for one EdgeConv layer; interleaves
            next-layer transpose/uv prep two tiles behind the selection."""
            l3 = layer == 3
            vslice = v_drams[D]
            with tc.tile_pool(name=f"L{layer}d", bufs=1, space="PSUM") as dps, \
                 tc.tile_pool(name=f"L{layer}w", bufs=1, space="PSUM") as wps, \
                 tc.tile_pool(name=f"L{layer}dd", bufs=1) as ddp, \
                 tc.tile_pool(name=f"L{layer}sel", bufs=2) as selp, \
                 tc.tile_pool(name=f"L{layer}g", bufs=3) as gp:
                pending = []

                def flush_pending():
                    gb_p, t_p = pending.pop(0)
                    vm = gp.tile([128, D], F32, name="vm")
                    nc.vector.tensor_reduce(out=vm[:],
                                            in_=gb_p.rearrange("p m d -> p d m"),
                                            axis=AX.X, op=ALU.max)
                    nc.vector.tensor_tensor(out=h[:, t_p, :], in0=u[:, t_p, :],
                                            in1=vm[:], op=ALU.add)
                    if nxt is not None:
                        emit_next_prep(t_p, h, nxt)

                for t in range(NT):
                    dp = dps.tile([128, N], F32, name="dp")
                    tchunk = t // 4
                    for j in range(JC):
                        last = (j != tchunk)
                        nc.tensor.matmul(dp[:, ts(j, 512)], hA[0:C, ts(t, 128)],
                                         hA[0:C, ts(j, 512)],
                                         start=True, stop=False)
                        nc.tensor.matmul(dp[:, ts(j, 512)], ones_row,
                                         nsq_t[:, ts(j, 512)],
                                         start=False, stop=last)
                        if not last:
                            nc.tensor.matmul(dp[:, ts(t, 128)], idnN_sb[:],
                                             idnP_sb[:], start=False, stop=True,
                                             skip_group_check=True)
                    dde = ddp.tile([128, N], F32, name="dde")
                    stt_imm(dde[:].bitcast(U32), dp[:].bitcast(U32),
                            0xFFFFF800, iot[:])

                    cand = selp.tile([128, 128], F32, name="cand")
                    for s in range(16):
                        nc.vector.max(out=cand[:, ts(s, 8)],
                                      in_=dde[:, ts(s, 128)])
                    m1 = selp.tile([128, 8], F32, name="m1")
                    m2 = selp.tile([128, 8], F32, name="m2")
                    m3 = selp.tile([128, 8], F32, name="m3")
                    cw = selp.tile([128, 128], F32, name="cw")
                    cw2 = selp.tile([128, 128], F32, name="cw2")
                    nc.vector.max(out=m1[:], in_=cand[:])
                    nc.vector.match_replace(out=cw[:], in_to_replace=m1[:],
                                            in_values=cand[:], imm_value=-1e30)
                    nc.vector.max(out=m2[:], in_=cw[:])
                    nc.vector.match_replace(out=cw2[:], in_to_replace=m2[:],
                                            in_values=cw[:], imm_value=-1e30)
                    nc.vector.max(out=m3[:], in_=cw2[:])

                    jgu = selp.tile([128, 24], U32, name="jgu")
                    ts_imm(jgu[:, 0:8], m1[:].bitcast(U32), 0x7FF)
                    ts_imm(jgu[:, 8:16], m2[:].bitcast(U32), 0x7FF)
                    ts_imm(jgu[:, 16:24], m3[:].bitcast(U32), 0x7FF)
                    jg = selp.tile([128, 20], F16, name="jg")
                    nc.vector.tensor_copy(jg[:], jgu[:, 0:20])

                    wp = wps.tile([128, 8, 20], F32, name="wp")
                    for c in range(8):
                        nc.tensor.matmul(wp[:, c, :], prm[:, c, :], jg[:],
                                         start=True, stop=True)
                    wrapped = selp.tile([128, 160], I16, name="wrapped")
                    nc.vector.tensor_copy(
                        wrapped[:].rearrange("q (s c) -> q s c", c=8),
                        wp[:].rearrange("q c s -> q s c"))

                    gb = gp.tile([128, 20, D], F32, name="gb")
                    q0 = (3 * t) % 4
                    nc.gpsimd.dma_gather(gb[:, 0:8, :], vslice,
                                         wrapped[:, 0:64], 1024, 1024, D,
                                         queue_num=q0)
                    nc.gpsimd.dma_gather(gb[:, 8:16, :], vslice,
                                         wrapped[:, 64:128], 1024, 1024, D,
                                         queue_num=(q0 + 1) % 4)
                    nc.gpsimd.dma_gather(gb[:, 16:20, :], vslice,
                                         wrapped[:, 128:160], 512, 512, D,
                                         queue_num=(q0 + 2) % 4)
                    pending.append((gb, t))
                    if len(pending) > 2:
                        flush_pending()
                while pending:
                    flush_pending()
            if nxt is not None:
                finalize_nsq(nxt)

        # ---------------- layer drivers ----------------
        hA3_sb = fp.tile([128, N], F32R)
        xsq_sh = fp.tile([128, N], F32R)
        h3 = fp.tile([128, NT, 256], F32)
        u3 = fp.tile([128, NT, 256], F32)

        with tc.tile_pool(name="lay12", bufs=1) as lp12:
            hA1_sb = lp12.tile([4, N], F32R)
            nc.sync.dma_start(hA1_sb[:], hA1)
            h1 = lp12.tile([128, NT, 64], F32)
            u1 = lp12.tile([128, NT, 64], F32)
            hA2_sb = lp12.tile([65, N], F32R)
            h2 = lp12.tile([128, NT, 128], F32)
            u2 = lp12.tile([128, NT, 128], F32)
            nc.sync.dma_start(hA2_sb[64:65, :], onesr)

            for t in range(NT):
                emit_uv(t, hA1_sb, 3, 64, w["ABc1"], w["BB1"], u1,
                        v_drams[64], False)

            nxt2 = dict(C=64, D=128, hA=hA2_sb,
                        xsq=xsq_sh, ABc=w["ABc2"], BB=w["BB2"], u=u2,
                        vD=v_drams[128], nsqrow=nsq2_sb, l3=False)
            run_layer(tc, 1, 3, 64, hA1_sb, nsq1_sb, onesSB[0:1, :], u1, h1,
                      nxt2)

            nxt3 = dict(C=128, D=256, hA=hA3_sb,
                        xsq=xsq_sh, ABc=w["AB3"], BB=w["BB3"], u=u3,
                        vD=v_drams[256], nsqrow=nsq3, l3=True)
            run_layer(tc, 2, 64, 128, hA2_sb, nsq2_sb, onesSB[32:33, :], u2,
                      h2, nxt3)

        run_layer(tc, 3, 128, 256, hA3_sb, nsq3, onesSB[64:65, :], u3, h3,
                  None)

        # ---------- global max pool + FC head ----------
        with tc.tile_pool(name="head", bufs=1) as hp, \
             tc.tile_pool(name="headps", bufs=1, space="PSUM") as hps:
            gmax = hp.tile([128, 256], F32)
            nc.vector.tensor_reduce(out=gmax[:],
                                    in_=h3.rearrange("p g d -> p d g"),
                                    axis=AX.X, op=ALU.max)
            g0 = hp.tile([128, 1], F32)
            g1 = hp.tile([128, 1], F32)
            for half, gdst in ((0, g0), (1, g1)):
                tp = hps.tile([128, 128], F32, name="tp", tag="tp")
                nc.tensor.transpose(tp[:], gmax[:, ts(half, 128)], idn_sb[:])
                tsb = hp.tile([128, 128], F32, name=f"tsb_{half}")
                nc.scalar.copy(tsb[:], tp[:])
                nc.vector.tensor_reduce(out=gdst[:], in_=tsb[:], axis=AX.X,
                                        op=ALU.max)

            fw1 = [hp.tile([128, 512], F32, name=f"fw1_{kk}") for kk in range(2)]
            fw2 = [hp.tile([128, 256], F32, name=f"fw2_{kk}") for kk in range(4)]
            fw3 = [hp.tile([128, 16], F32, name=f"fw3_{kk}") for kk in range(2)]
            fb1 = hp.tile([128, 4], F32)
            fb2 = hp.tile([128, 2], F32)
            fb3 = hp.tile([16, 1], F32)
            for kk in range(2):
                nc.sync.dma_start(fw1[kk][:], fc1w[ts(kk, 128), :])
                nc.sync.dma_start(fw3[kk][:], fc3w[ts(kk, 128), :])
            for kk in range(4):
                nc.sync.dma_start(fw2[kk][:], fc2w[ts(kk, 128), :])
            nc.sync.dma_start(fb1[:], fc1b)
            nc.sync.dma_start(fb2[:], fc2b)
            nc.sync.dma_start(fb3[:], fc3b)

            a1 = [hp.tile([128, 1], F32, name=f"a1_{m}") for m in range(4)]
            for m in range(4):
                p = hps.tile([128, 1], F32, name="fcp", tag="fcp")
                nc.tensor.matmul(p[:], fw1[0][:, ts(m, 128)], g0[:],
                                 start=True, stop=False)
                nc.tensor.matmul(p[:], fw1[1][:, ts(m, 128)], g1[:],
                                 start=False, stop=True)
                nc.scalar.activation(a1[m][:], p[:], AF.Relu,
                                     bias=fb1[:, m:m + 1], scale=1.0)
            a2 = [hp.tile([128, 1], F32, name=f"a2_{m}") for m in range(2)]
            for m in range(2):
                p = hps.tile([128, 1], F32, name="fcp", tag="fcp")
                for kk in range(4):
                    nc.tensor.matmul(p[:], fw2[kk][:, ts(m, 128)], a1[kk][:],
                                     start=(kk == 0), stop=(kk == 3))
                nc.scalar.activation(a2[m][:], p[:], AF.Relu,
                                     bias=fb2[:, m:m + 1], scale=1.0)
            p3 = hps.tile([128, 1], F32, name="fcp", tag="fcp")[0:16, :]
            for kk in range(2):
                nc.tensor.matmul(p3[:], fw3[kk][:], a2[kk][:],
                                 start=(kk == 0), stop=(kk == 1))
            o_sb = hp.tile([16, 1], F32)
            nc.scalar.activation(o_sb[:], p3[:], AF.Identity, bias=fb3[:],
                                 scale=1.0)
            nc.sync.dma_start(out, o_sb[:])

    nc.finalize()
    return nc


def get_nc():
    if 0 not in _NC_CACHE:
        _NC_CACHE[0] = _builder()
    return _NC_CACHE[0]


def make_in_maps(x, W1, b1, W2, b2, W3, b3, fc1_w, fc1_b, fc2_w, fc2_b,
                 fc3_w, fc3_b):
    f32 = np.float32
    x = np.asarray(x, f32)
    B = x.shape[0]
    W1, W2, W3 = np.asarray(W1, f32), np.asarray(W2, f32), np.asarray(W3, f32)
    prm = np.zeros((128, 8, 128), dtype=np.float16)
    for c in range(8):
        for j in range(128):
            prm[16 * c + (j % 16), c, j] = 1.0
    shared = {
        "ABc1": np.concatenate([W1[:3] - W1[3:6], np.asarray(b1, f32)[None]], 0),
        "BB1": np.ascontiguousarray(W1[3:6]),
        "ABc2": np.concatenate([W2[:64] - W2[64:], np.asarray(b2, f32)[None]], 0),
        "BB2": np.ascontiguousarray(W2[64:]),
        "AB3": np.ascontiguousarray(W3[:128] - W3[128:]),
        "BB3": np.ascontiguousarray(W3[128:]),
        "b3r": np.asarray(b3, f32)[None],
        "idn": np.eye(128, dtype=f32),
        "idnN": (np.eye(128) * -1e30).astype(f32),
        "idnP": np.eye(128, dtype=f32),
        "onesr": np.ones((1, N), f32),
        "onescol": np.ones((128, 1), f32),
        "iot": np.broadcast_to(np.arange(N, dtype=np.uint32), (128, N)).copy(),
        "prm": prm,
        "fc1w": np.asarray(fc1_w, f32),
        "fc1b": np.ascontiguousarray(np.asarray(fc1_b, f32).reshape(4, 128).T),
        "fc2w": np.asarray(fc2_w, f32),
        "fc2b": np.ascontiguousarray(np.asarray(fc2_b, f32).reshape(2, 128).T),
        "fc3w": np.pad(np.asarray(fc3_w, f32), ((0, 0), (0, 6))),
        "fc3b": np.pad(np.asarray(fc3_b, f32), (0, 6))[:, None],
    }
    in_maps = []
    for bb in range(B):
        xb = x[bb]
        xT = np.ascontiguousarray(xb.T)
        nsq = -(xb * xb).sum(-1)[None, :].astype(f32)
        m = dict(shared)
        m["hA1"] = np.concatenate([xT, np.ones((1, N), f32)], 0)
        m["nsq1"] = 0.5 * nsq
        in_maps.append(m)
    return in_maps


def kernel(x, k, W1, b1, W2, b2, W3, b3, fc1_w, fc1_b, fc2_w, fc2_b, fc3_w,
           fc3_b):
    from concourse import bass_utils
    x = np.asarray(x)
    assert int(k) == 20 and x.shape[1] == N and x.shape[2] == 3
    B = x.shape[0]
    assert B == 8
    nc = get_nc()
    in_maps = make_in_maps(x, W1, b1, W2, b2, W3, b3,
                           fc1_w, fc1_b, fc2_w, fc2_b, fc3_w, fc3_b)
    res = bass_utils.run_bass_kernel_spmd(nc, in_maps, core_ids=list(range(B)))
    outs = np.stack([res.results[bb]["out"][:10, 0] for bb in range(B)], axis=0)
    return outs.astype(np.float32)


# revision 24
# speedup vs baseline: 1.4892x; 1.0989x over previous
"""DGCNN (3x DynamicEdgeConv + global max pool + MLP head) on 8 Trainium2
NeuronCores, data-parallel over the batch (one point cloud per core).

EdgeConv algebra: h_ij = [x_i, x_j - x_i] @ W + b = u_i + v_j with
  u = x @ (Wa - Wb) + b,  v = x @ Wb;  out_i = u_i + max_{j in knn(i)} v_j.

kNN key d''_ij = 2 x_i.x_j - |x_j|^2 (largest = nearest); the self column is
killed with a -1e30 diagonal matmul so the top-20 are exactly the neighbors.
Distances use fp32r matmuls; the contraction is augmented so one matmul per
512-col chunk computes 2x.x + nsq (layers 1/2).  Feature rows are stored as
hA = [nsq; feat; ones], hX = [ones; 2*feat] so u/v matmuls slice [feat; ones]
and never touch the late-computed nsq row, letting next-layer transposes and
u/v matmuls interleave into the current layer's selection loop.

The column index is embedded in the low 11 mantissa bits of each distance
(bitwise AND+OR with an iota row) so top-k selection needs no max_index: 16
segment max8's yield 128 candidates, 3x max8 + 2x match_replace pick the
top-24, and indices pop out of the winning values with a bitwise AND.
Neighbor v-rows are fetched with 3 batched dma_gather calls (1024+1024+512
rows) over 4 SWDGE queues; their int16 index list is built by 8 permutation
matmuls that transpose jtab into the gather's wrapped 16-partition layout.
The 20-way neighbor max is one DVE tensor_reduce (deferred 2 tiles to hide
gather latency), added to u to form the layer output.
"""
import numpy as np

_NC_CACHE = {}

N, NT, JC = 2048, 16, 4


def _builder():
    import concourse.bacc as bacc
    import concourse.mybir as mybir
    from concourse.tile import TileContext

    F32 = mybir.dt.float32
    F32R = mybir.dt.float32r
    F16 = mybir.dt.float16
    U32 = mybir.dt.uint32
    I16 = mybir.dt.int16
    AF = mybir.ActivationFunctionType
    ALU = mybir.AluOpType
    AX = mybir.AxisListType

    def ts(i, s):
        return slice(i * s, (i + 1) * s)

    nc = bacc.Bacc("TRN2", num_devices=8, num_swdge_queues=4)

    def din(name, shape, dt=F32R):
        return nc.dram_tensor(name, shape, dt, kind="ExternalInput").ap()

    hA1 = din("hA1", [4, N])            # [x^T; ones]
    nsq1 = din("nsq1", [1, N])          # -0.5 |x|^2
    ABc1 = din("ABc1", [4, 64])         # [W1a-W1b; b1]
    BB1 = din("BB1", [3, 64])
    ABc2 = din("ABc2", [65, 128])       # [W2a-W2b; b2]
    BB2 = din("BB2", [64, 128])
    AB3 = din("AB3", [128, 256])
    BB3 = din("BB3", [128, 256])
    b3r = din("b3r", [1, 256])
    idn = din("idn", [128, 128], F32)
    idnN = din("idnN", [128, 128])      # -1e30 * I
    idnP = din("idnP", [128, 128])      # I
    onesr = din("onesr", [1, N])        # ones row
    onescol = din("onescol", [128, 1])  # ones column
    iot_in = din("iot", [128, N], U32)
    prm_in = din("prm", [128, 8, 128], F16)
    fc1w = din("fc1w", [256, 512], F32)
    fc1b = din("fc1b", [128, 4], F32)
    fc2w = din("fc2w", [512, 256], F32)
    fc2b = din("fc2b", [128, 2], F32)
    fc3w = din("fc3w", [256, 16], F32)
    fc3b = din("fc3b", [16, 1], F32)
    out = nc.dram_tensor("out", [16, 1], F32, kind="ExternalOutput").ap()

    def ts_imm(out_ap, in0, imm):
        eng = nc.vector
        return eng.add_instruction(
            mybir.InstTensorScalarPtr(
                name=eng.bass.get_next_instruction_name(),
                op0=ALU.bitwise_and, op1=ALU.bypass,
                ins=[eng.lower_ap(in0),
                     mybir.ImmediateValue(dtype=U32, value=imm)],
                outs=[eng.lower_ap(out_ap)]))

    def stt_imm(out_ap, in0, imm, in1):
        eng = nc.vector
        return eng.add_instruction(
            mybir.InstTensorScalarPtr(
                name=eng.bass.get_next_instruction_name(),
                is_scalar_tensor_tensor=True,
                op0=ALU.bitwise_and, op1=ALU.bitwise_or,
                ins=[eng.lower_ap(in0),
                     mybir.ImmediateValue(dtype=U32, value=imm),
                     eng.lower_ap(in1)],
                outs=[eng.lower_ap(out_ap)]))

    from contextlib import ExitStack
    with TileContext(nc) as tc, ExitStack() as stack:
        cp = stack.enter_context(tc.tile_pool(name="const", bufs=1))
        fp = stack.enter_context(tc.tile_pool(name="feat", bufs=1))
        vdp = stack.enter_context(tc.tile_pool(name="vdram", bufs=1, space="DRAM"))
        uvps = stack.enter_context(tc.tile_pool(name="uvps", bufs=1, space="PSUM"))
        tps = stack.enter_context(tc.tile_pool(name="tps", bufs=1, space="PSUM"))
        uvsb = stack.enter_context(tc.tile_pool(name="uvsb", bufs=4))

        v_drams = {}
        for _D in (64, 128, 256):
            v_drams[_D] = vdp.tile([N, _D], F32, name=f"v_dram{_D}")
        idn_sb = cp.tile([128, 128], F32)
        nc.sync.dma_start(idn_sb[:], idn)
        idnN_sb = cp.tile([128, 128], F32R)
        nc.sync.dma_start(idnN_sb[:], idnN)
        idnP_sb = cp.tile([128, 128], F32R)
        nc.sync.dma_start(idnP_sb[:], idnP)
        onesSB = cp.tile([65, 128], F32R)
        nc.sync.dma_start(onesSB[0:1, :], onesr[0:1, 0:128])
        nc.sync.dma_start(onesSB[32:33, :], onesr[0:1, 0:128])
        nc.sync.dma_start(onesSB[64:65, :], onesr[0:1, 0:128])
        nsqall = cp.tile([65, N], F32R)
        onescol_sb = cp.tile([128, 1], F32R)
        nc.sync.dma_start(onescol_sb[:], onescol)
        iot = cp.tile([128, N], U32)
        nc.sync.dma_start(iot[:], iot_in)
        prm = cp.tile([128, 8, 128], F16)
        nc.sync.dma_start(prm[:], prm_in)
        ones1 = onesSB[0:1, 0:128]
        nsq1_sb = nsqall[0:1, :]
        nsq2_sb = nsqall[32:33, :]
        nsq3 = nsqall[64:65, :]
        nc.sync.dma_start(nsq1_sb, nsq1)

        w = {}
        for nm, ap_, shape in [("ABc1", ABc1, [4, 64]), ("BB1", BB1, [3, 64]),
                               ("ABc2", ABc2, [65, 128]), ("BB2", BB2, [64, 128]),
                               ("AB3", AB3, [128, 256]), ("BB3", BB3, [128, 256]),
                               ("b3r", b3r, [1, 256])]:
            t_ = cp.tile(shape, F32R, name=f"w_{nm}")
            nc.sync.dma_start(t_[:], ap_)
            w[nm] = t_

        def emit_uv(t, hA, C, D, ABc, BB, u, vslice, l3):
            """u/v matmuls for tile t; hA rows [nsq(0); feat(1..C); ones(C+1)]
            (L3: hA = feat only, bias via ones1 @ b3r)."""
            pair = uvps.tile([128, 512], F32, name="uvpair")
            up = pair[:, 0:D]
            vp = pair[:, 256:256 + D]
            if l3:
                nc.tensor.matmul(vp, hA[:, ts(t, 128)], BB[:],
                                 start=True, stop=True, skip_group_check=True)
                nc.tensor.matmul(up, hA[:, ts(t, 128)], ABc[:],
                                 start=True, stop=False, skip_group_check=True)
                nc.tensor.matmul(up, ones1, w["b3r"][:],
                                 start=False, stop=True, skip_group_check=True)
            else:
                nc.tensor.matmul(vp, hA[0:C, ts(t, 128)], BB[:],
                                 start=True, stop=True, skip_group_check=True)
                nc.tensor.matmul(up, hA[0:C + 1, ts(t, 128)], ABc[:],
                                 start=True, stop=True, skip_group_check=True)
            vsb = uvsb.tile([128, 256], F32, name="vsb")[:, 0:D]
            nc.scalar.copy(vsb, vp)
            nc.scalar.copy(u[:, t, :], up)
            nc.sync.dma_start(vslice[ts(t, 128), :], vsb)

        def emit_next_prep(t, h, nxt):
            """Transpose h tile t into next layer's hA/hX (+per-tile square)."""
            C2 = nxt["C"]
            tp = tps.tile([128, 128], F32, name="tp", tag="tp")[0:C2, :]
            nc.tensor.transpose(tp, h[:, t, 0:C2], idn_sb[:])
            nc.scalar.activation(nxt["hA"][0:C2, ts(t, 128)], tp, AF.Copy,
                                 scale=1.0)
            nc.scalar.square(nxt["xsq"][0:C2, ts(t, 128)], tp)
            emit_uv(t, nxt["hA"], C2, nxt["D"], nxt["ABc"], nxt["BB"],
                    nxt["u"], nxt["vD"], nxt["l3"])

        def finalize_nsq(nxt):
            C2 = nxt["C"]
            for j in range(JC):
                sqp = tps.tile([1, 512], F32, name="sqp", tag="sqp")
                nc.tensor.matmul(sqp[:], onescol_sb[0:C2, :],
                                 nxt["xsq"][0:C2, ts(j, 512)],
                                 start=True, stop=True)
                nc.scalar.activation(nxt["nsqrow"][0:1, ts(j, 512)], sqp[:],
                                     AF.Copy, scale=-0.5)

        def run_layer(tc, layer, C, D, hA, nsq_t, ones_row, u, h, nxt):
            """Selection + gather + reduce for one EdgeConv layer; interleaves
            next-layer transpose/uv prep two tiles behind the selection."""
            l3 = layer == 3
            vslice = v_drams[D]
            with tc.tile_pool(name=f"L{layer}d", bufs=1, space="PSUM") as dps, \
                 tc.tile_pool(name=f"L{layer}w", bufs=1, space="PSUM") as wps, \
                 tc.tile_pool(name=f"L{layer}dd", bufs=1) as ddp, \
                 tc.tile_pool(name=f"L{layer}sel", bufs=2) as selp, \
                 tc.tile_pool(name=f"L{layer}g", bufs=3) as gp:
                pending = []

                def flush_pending():
                    gb_p, t_p = pending.pop(0)
                    vm = gp.tile([128, D], F32, name="vm")
                    nc.vector.tensor_reduce(out=vm[:],
                                            in_=gb_p.rearrange("p m d -> p d m"),
                                            axis=AX.X, op=ALU.max)
                    nc.vector.tensor_tensor(out=h[:, t_p, :], in0=u[:, t_p, :],
                                            in1=vm[:], op=ALU.add)
                    if nxt is not None:
                        emit_next_prep(t_p, h, nxt)

                for t in range(NT):
                    dp = dps.tile([128, N], F32, name="dp")
                    tchunk = t // 4
                    for j in range(JC):
                        last = (j != tchunk)
                        nc.tensor.matmul(dp[:, ts(j, 512)], hA[0:C, ts(t, 128)],
                                         hA[0:C, ts(j, 512)],
                                         start=True, stop=False)
                        nc.tensor.matmul(dp[:, ts(j, 512)], ones_row,
                                         nsq_t[:, ts(j, 512)],
                                         start=False, stop=last)
                        if not last:
                            nc.tensor.matmul(dp[:, ts(t, 128)], idnN_sb[:],
                                             idnP_sb[:], start=False, stop=True,
                                             skip_group_check=True)
                    dde = ddp.tile([128, N], F32, name="dde")
                    stt_imm(dde[:].bitcast(U32), dp[:].bitcast(U32),
                            0xFFFFF800, iot[:])

                    cand = selp.tile([128, 96], F32, name="cand")
                    for s in range(12):
                        lo = 171 * s
                        hi = min(lo + 171, N)
                        nc.vector.max(out=cand[:, ts(s, 8)],
                                      in_=dde[:, lo:hi])
                    m1 = selp.tile([128, 8], F32, name="m1")
                    m2 = selp.tile([128, 8], F32, name="m2")
                    m3 = selp.tile([128, 8], F32, name="m3")
                    cw = selp.tile([128, 96], F32, name="cw")
                    cw2 = selp.tile([128, 96], F32, name="cw2")
                    nc.vector.max(out=m1[:], in_=cand[:])
                    nc.vector.match_replace(out=cw[:], in_to_replace=m1[:],
                                            in_values=cand[:], imm_value=-1e30)
                    nc.vector.max(out=m2[:], in_=cw[:])
                    nc.vector.match_replace(out=cw2[:], in_to_replace=m2[:],
                                            in_values=cw[:], imm_value=-1e30)
                    nc.vector.max(out=m3[:], in_=cw2[:])

                    jgu = selp.tile([128, 24], U32, name="jgu")
                    ts_imm(jgu[:, 0:8], m1[:].bitcast(U32), 0x7FF)
                    ts_imm(jgu[:, 8:16], m2[:].bitcast(U32), 0x7FF)
                    ts_imm(jgu[:, 16:24], m3[:].bitcast(U32), 0x7FF)
                    jg = selp.tile([128, 20], F16, name="jg")
                    nc.vector.tensor_copy(jg[:], jgu[:, 0:20])

                    wp = wps.tile([128, 8, 20], F32, name="wp")
                    for c in range(8):
                        nc.tensor.matmul(wp[:, c, :], prm[:, c, :], jg[:],
                                         start=True, stop=True)
                    if len(pending) >= 2:
                        flush_pending()
                    wrapped = selp.tile([128, 160], I16, name="wrapped")
                    nc.scalar.copy(
                        wrapped[:].rearrange("q (s c) -> q s c", c=8),
                        wp[:].rearrange("q c s -> q s c"))

                    gb = gp.tile([128, 20, D], F32, name="gb")
                    q0 = (3 * t) % 4
                    nc.gpsimd.dma_gather(gb[:, 0:8, :], vslice,
                                         wrapped[:, 0:64], 1024, 1024, D,
                                         queue_num=q0)
                    nc.gpsimd.dma_gather(gb[:, 8:16, :], vslice,
                                         wrapped[:, 64:128], 1024, 1024, D,
                                         queue_num=(q0 + 1) % 4)
                    nc.gpsimd.dma_gather(gb[:, 16:20, :], vslice,
                                         wrapped[:, 128:160], 512, 512, D,
                                         queue_num=(q0 + 2) % 4)
                    pending.append((gb, t))
                while pending:
                    flush_pending()
            if nxt is not None:
                finalize_nsq(nxt)

        # ---------------- layer drivers ----------------
        hA3_sb = fp.tile([128, N], F32R)
        xsq_sh = fp.tile([128, N], F32R)
        h3 = fp.tile([128, NT, 256], F32)
        u3 = fp.tile([128, NT, 256], F32)

        with tc.tile_pool(name="lay12", bufs=1) as lp12:
            hA1_sb = lp12.tile([4, N], F32R)
            nc.sync.dma_start(hA1_sb[:], hA1)
            h1 = lp12.tile([128, NT, 64], F32)
            u1 = lp12.tile([128, NT, 64], F32)
            hA2_sb = lp12.tile([65, N], F32R)
            h2 = lp12.tile([128, NT, 128], F32)
            u2 = lp12.tile([128, NT, 128], F32)
            nc.sync.dma_start(hA2_sb[64:65, :], onesr)

            for t in range(NT):
                emit_uv(t, hA1_sb, 3, 64, w["ABc1"], w["BB1"], u1,
                        v_drams[64], False)

            nxt2 = dict(C=64, D=128, hA=hA2_sb,
                        xsq=xsq_sh, ABc=w["ABc2"], BB=w["BB2"], u=u2,
                        vD=v_drams[128], nsqrow=nsq2_sb, l3=False)
            run_layer(tc, 1, 3, 64, hA1_sb, nsq1_sb, onesSB[0:1, :], u1, h1,
                      nxt2)

            nxt3 = dict(C=128, D=256, hA=hA3_sb,
                        xsq=xsq_sh, ABc=w["AB3"], BB=w["BB3"], u=u3,
                        vD=v_drams[256], nsqrow=nsq3, l3=True)
            run_layer(tc, 2, 64, 128, hA2_sb, nsq2_sb, onesSB[32:33, :], u2,
                      h2, nxt3)

        run_layer(tc, 3, 128, 256, hA3_sb, nsq3, onesSB[64:65, :], u3, h3,
                  None)

        # ---------- global max pool + FC head ----------
        with tc.tile_pool(name="head", bufs=1) as hp, \
             tc.tile_pool(name="headps", bufs=1, space="PSUM") as hps:
            gmax = hp.tile([128, 256], F32)
            nc.vector.tensor_reduce(out=gmax[:],
                                    in_=h3.rearrange("p g d -> p d g"),
                                    axis=AX.X, op=ALU.max)
            g0 = hp.tile([128, 1], F32)
            g1 = hp.tile([128, 1], F32)
            for half, gdst in ((0, g0), (1, g1)):
                tp = hps.tile([128, 128], F32, name="tp", tag="tp")
                nc.tensor.transpose(tp[:], gmax[:, ts(half, 128)], idn_sb[:])
                tsb = hp.tile([128, 128], F32, name=f"tsb_{half}")
                nc.scalar.copy(tsb[:], tp[:])
                nc.vector.tensor_reduce(out=gdst[:], in_=tsb[:], axis=AX.X,
                                        op=ALU.max)

            fw1 = [hp.tile([128, 512], F32, name=f"fw1_{kk}") for kk in range(2)]
            fw2 = [hp.tile([128, 256], F32, name=f"fw2_{kk}") for kk in range(4)]
            fw3 = [hp.tile([128, 16], F32, name=f"fw3_{kk}") for kk in range(2)]
            fb1 = hp.tile([128, 4], F32)
            fb2 = hp.tile([128, 2], F32)
            fb3 = hp.tile([16, 1], F32)
            for kk in range(2):
                nc.sync.dma_start(fw1[kk][:], fc1w[ts(kk, 128), :])
                nc.sync.dma_start(fw3[kk][:], fc3w[ts(kk, 128), :])
            for kk in range(4):
                nc.sync.dma_start(fw2[kk][:], fc2w[ts(kk, 128), :])
            nc.sync.dma_start(fb1[:], fc1b)
            nc.sync.dma_start(fb2[:], fc2b)
            nc.sync.dma_start(fb3[:], fc3b)

            a1 = [hp.tile([128, 1], F32, name=f"a1_{m}") for m in range(4)]
            for m in range(4):
                p = hps.tile([128, 1], F32, name="fcp", tag="fcp")
                nc.tensor.matmul(p[:], fw1[0][:, ts(m, 128)], g0[:],
                                 start=True, stop=False)
                nc.tensor.matmul(p[:], fw1[1][:, ts(m, 128)], g1[:],
                                 start=False, stop=True)
                nc.scalar.activation(a1[m][:], p[:], AF.Relu,
                                     bias=fb1[:, m:m + 1], scale=1.0)
            a2 = [hp.tile([128, 1], F32, name=f"a2_{m}") for m in range(2)]
            for m in range(2):
                p = hps.tile([128, 1], F32, name="fcp", tag="fcp")
                for kk in range(4):
                    nc.tensor.matmul(p[:], fw2[kk][:, ts(m, 128)], a1[kk][:],
                                     start=(kk == 0), stop=(kk == 3))
                nc.scalar.activation(a2[m][:], p[:], AF.Relu,
                                     bias=fb2[:, m:m + 1], scale=1.0)
            p3 = hps.tile([128, 1], F32, name="fcp", tag="fcp")[0:16, :]
            for kk in range(2):
                nc.tensor.matmul(p3[:], fw3[kk][:], a2[kk][:],
                                 start=(kk == 0), stop=(kk == 1))
            o_sb = hp.tile([16, 1], F32)
            nc.scalar.activation(o_sb[:], p3[:], AF.Identity, bias=fb3[:],
                                 scale=1.0)
            nc.sync.dma_start(out, o_sb[:])

    nc.finalize()
    return nc


def get_nc():
    if 0 not in _NC_CACHE:
        _NC_CACHE[0] = _builder()
    return _NC_CACHE[0]


def make_in_maps(x, W1, b1, W2, b2, W3, b3, fc1_w, fc1_b, fc2_w, fc2_b,
                 fc3_w, fc3_b):
    f32 = np.float32
    x = np.asarray(x, f32)
    B = x.shape[0]
    W1, W2, W3 = np.asarray(W1, f32), np.asarray(W2, f32), np.asarray(W3, f32)
    prm = np.zeros((128, 8, 128), dtype=np.float16)
    for c in range(8):
        for j in range(128):
            prm[16 * c + (j % 16), c, j] = 1.0
    shared = {
        "ABc1": np.concatenate([W1[:3] - W1[3:6], np.asarray(b1, f32)[None]], 0),
        "BB1": np.ascontiguousarray(W1[3:6]),
        "ABc2": np.concatenate([W2[:64] - W2[64:], np.asarray(b2, f32)[None]], 0),
        "BB2": np.ascontiguousarray(W2[64:]),
        "AB3": np.ascontiguousarray(W3[:128] - W3[128:]),
        "BB3": np.ascontiguousarray(W3[128:]),
        "b3r": np.asarray(b3, f32)[None],
        "idn": np.eye(128, dtype=f32),
        "idnN": (np.eye(128) * -1e30).astype(f32),
        "idnP": np.eye(128, dtype=f32),
        "onesr": np.ones((1, N), f32),
        "onescol": np.ones((128, 1), f32),
        "iot": np.broadcast_to(np.arange(N, dtype=np.uint32), (128, N)).copy(),
        "prm": prm,
        "fc1w": np.asarray(fc1_w, f32),
        "fc1b": np.ascontiguousarray(np.asarray(fc1_b, f32).reshape(4, 128).T),
        "fc2w": np.asarray(fc2_w, f32),
        "fc2b": np.ascontiguousarray(np.asarray(fc2_b, f32).reshape(2, 128).T),
        "fc3w": np.pad(np.asarray(fc3_w, f32), ((0, 0), (0, 6))),
        "fc3b": np.pad(np.asarray(fc3_b, f32), (0, 6))[:, None],
    }
    in_maps = []
    for bb in range(B):
        xb = x[bb]
        xT = np.ascontiguousarray(xb.T)
        nsq = -(xb * xb).sum(-1)[None, :].astype(f32)
        m = dict(shared)
        m["hA1"] = np.concatenate([xT, np.ones((1, N), f32)], 0)
        m["nsq1"] = 0.5 * nsq
        in_maps.append(m)
    return in_maps


def kernel(x, k, W1, b1, W2, b2, W3, b3, fc1_w, fc1_b, fc2_w, fc2_b, fc3_w,
           fc3_b):
    from concourse import bass_utils
    x = np.asarray(x)
    assert int(k) == 20 and x.shape[1] == N and x.shape[2] == 3
    B = x.shape[0]
    assert B == 8
    nc = get_nc()
    in_maps = make_in_maps(x, W1, b1, W2, b2, W3, b3,
                           fc1_w, fc1_b, fc2_w, fc2_b, fc3_w, fc3_b)
    res = bass_utils.run_bass_kernel_spmd(nc, in_maps, core_ids=list(range(B)))
    outs = np.stack([res.results[bb]["out"][:10, 0] for bb in range(B)], axis=0)
    return outs.astype(np.float32)


# revision 25
# speedup vs baseline: 1.5163x; 1.0182x over previous
"""DGCNN (3x DynamicEdgeConv + global max pool + MLP head) on 8 Trainium2
NeuronCores, data-parallel over the batch (one point cloud per core).

EdgeConv algebra: h_ij = [x_i, x_j - x_i] @ W + b = u_i + v_j with
  u = x @ (Wa - Wb) + b,  v = x @ Wb;  out_i = u_i + max_{j in knn(i)} v_j.

kNN key d''_ij = 2 x_i.x_j - |x_j|^2 (largest = nearest); the self column is
killed with a -1e30 diagonal matmul so the top-20 are exactly the neighbors.
Distances use fp32r matmuls; the contraction is augmented so one matmul per
512-col chunk computes 2x.x + nsq (layers 1/2).  Feature rows are stored as
hA = [nsq; feat; ones], hX = [ones; 2*feat] so u/v matmuls slice [feat; ones]
and never touch the late-computed nsq row, letting next-layer transposes and
u/v matmuls interleave into the current layer's selection loop.

The column index is embedded in the low 11 mantissa bits of each distance
(bitwise AND+OR with an iota row) so top-k selection needs no max_index: 16
segment max8's yield 128 candidates, 3x max8 + 2x match_replace pick the
top-24, and indices pop out of the winning values with a bitwise AND.
Neighbor v-rows are fetched with 3 batched dma_gather calls (1024+1024+512
rows) over 4 SWDGE queues; their int16 index list is built by 8 permutation
matmuls that transpose jtab into the gather's wrapped 16-partition layout.
The 20-way neighbor max is one DVE tensor_reduce (deferred 2 tiles to hide
gather latency), added to u to form the layer output.
"""
import numpy as np

_NC_CACHE = {}

N, NT, JC = 2048, 16, 4


def _builder():
    import concourse.bacc as bacc
    import concourse.mybir as mybir
    from concourse.tile import TileContext

    F32 = mybir.dt.float32
    F32R = mybir.dt.float32r
    F16 = mybir.dt.float16
    U32 = mybir.dt.uint32
    I16 = mybir.dt.int16
    AF = mybir.ActivationFunctionType
    ALU = mybir.AluOpType
    AX = mybir.AxisListType

    def ts(i, s):
        return slice(i * s, (i + 1) * s)

    nc = bacc.Bacc("TRN2", num_devices=8, num_swdge_queues=4)

    def din(name, shape, dt=F32R):
        return nc.dram_tensor(name, shape, dt, kind="ExternalInput").ap()

    hA1 = din("hA1", [4, N])            # [x^T; ones]
    nsq1 = din("nsq1", [1, N])          # -0.5 |x|^2
    ABc1 = din("ABc1", [4, 64])         # [W1a-W1b; b1]
    BB1 = din("BB1", [3, 64])
    ABc2 = din("ABc2", [65, 128])       # [W2a-W2b; b2]
    BB2 = din("BB2", [64, 128])
    AB3 = din("AB3", [128, 256])
    BB3 = din("BB3", [128, 256])
    b3r = din("b3r", [1, 256])
    idn = din("idn", [128, 128], F32)
    idnN = din("idnN", [128, 128])      # -1e30 * I
    idnP = din("idnP", [128, 128])      # I
    onesr = din("onesr", [1, N])        # ones row
    onescol = din("onescol", [128, 1])  # ones column
    iot_in = din("iot", [128, N], U32)
    prm_in = din("prm", [128, 8, 128], F16)
    fc1w = din("fc1w", [256, 512], F32)
    fc1b = din("fc1b", [128, 4], F32)
    fc2w = din("fc2w", [512, 256], F32)
    fc2b = din("fc2b", [128, 2], F32)
    fc3w = din("fc3w", [256, 16], F32)
    fc3b = din("fc3b", [16, 1], F32)
    out = nc.dram_tensor("out", [16, 1], F32, kind="ExternalOutput").ap()

    def ts_imm(out_ap, in0, imm):
        eng = nc.vector
        return eng.add_instruction(
            mybir.InstTensorScalarPtr(
                name=eng.bass.get_next_instruction_name(),
                op0=ALU.bitwise_and, op1=ALU.bypass,
                ins=[eng.lower_ap(in0),
                     mybir.ImmediateValue(dtype=U32, value=imm)],
                outs=[eng.lower_ap(out_ap)]))

    def stt_imm(out_ap, in0, imm, in1):
        eng = nc.vector
        return eng.add_instruction(
            mybir.InstTensorScalarPtr(
                name=eng.bass.get_next_instruction_name(),
                is_scalar_tensor_tensor=True,
                op0=ALU.bitwise_and, op1=ALU.bitwise_or,
                ins=[eng.lower_ap(in0),
                     mybir.ImmediateValue(dtype=U32, value=imm),
                     eng.lower_ap(in1)],
                outs=[eng.lower_ap(out_ap)]))

    from contextlib import ExitStack
    with TileContext(nc) as tc, ExitStack() as stack:
        cp = stack.enter_context(tc.tile_pool(name="const", bufs=1))
        fp = stack.enter_context(tc.tile_pool(name="feat", bufs=1))
        vdp = stack.enter_context(tc.tile_pool(name="vdram", bufs=1, space="DRAM"))
        uvps = stack.enter_context(tc.tile_pool(name="uvps", bufs=1, space="PSUM"))
        tps = stack.enter_context(tc.tile_pool(name="tps", bufs=1, space="PSUM"))
        uvsb = stack.enter_context(tc.tile_pool(name="uvsb", bufs=4))

        v_drams = {}
        BF16 = mybir.dt.bfloat16
        v_dt = {64: F32, 128: BF16, 256: BF16}
        for _D in (64, 128, 256):
            v_drams[_D] = vdp.tile([N, _D], v_dt[_D], name=f"v_dram{_D}")
        idn_sb = cp.tile([128, 128], F32)
        nc.sync.dma_start(idn_sb[:], idn)
        idnN_sb = cp.tile([128, 128], F32R)
        nc.sync.dma_start(idnN_sb[:], idnN)
        idnP_sb = cp.tile([128, 128], F32R)
        nc.sync.dma_start(idnP_sb[:], idnP)
        onesSB = cp.tile([65, 128], F32R)
        nc.sync.dma_start(onesSB[0:1, :], onesr[0:1, 0:128])
        nc.sync.dma_start(onesSB[32:33, :], onesr[0:1, 0:128])
        nc.sync.dma_start(onesSB[64:65, :], onesr[0:1, 0:128])
        nsqall = cp.tile([65, N], F32R)
        onescol_sb = cp.tile([128, 1], F32R)
        nc.sync.dma_start(onescol_sb[:], onescol)
        iot = cp.tile([128, N], U32)
        nc.sync.dma_start(iot[:], iot_in)
        prm = cp.tile([128, 8, 128], F16)
        nc.sync.dma_start(prm[:], prm_in)
        ones1 = onesSB[0:1, 0:128]
        nsq1_sb = nsqall[0:1, :]
        nsq2_sb = nsqall[32:33, :]
        nsq3 = nsqall[64:65, :]
        nc.sync.dma_start(nsq1_sb, nsq1)

        w = {}
        for nm, ap_, shape in [("ABc1", ABc1, [4, 64]), ("BB1", BB1, [3, 64]),
                               ("ABc2", ABc2, [65, 128]), ("BB2", BB2, [64, 128]),
                               ("AB3", AB3, [128, 256]), ("BB3", BB3, [128, 256]),
                               ("b3r", b3r, [1, 256])]:
            t_ = cp.tile(shape, F32R, name=f"w_{nm}")
            nc.sync.dma_start(t_[:], ap_)
            w[nm] = t_

        def emit_uv(t, hA, C, D, ABc, BB, u, vslice, l3):
            """u/v matmuls for tile t; hA rows [nsq(0); feat(1..C); ones(C+1)]
            (L3: hA = feat only, bias via ones1 @ b3r)."""
            pair = uvps.tile([128, 512], F32, name="uvpair")
            up = pair[:, 0:D]
            vp = pair[:, 256:256 + D]
            if l3:
                nc.tensor.matmul(vp, hA[:, ts(t, 128)], BB[:],
                                 start=True, stop=True, skip_group_check=True)
                nc.tensor.matmul(up, hA[:, ts(t, 128)], ABc[:],
                                 start=True, stop=False, skip_group_check=True)
                nc.tensor.matmul(up, ones1, w["b3r"][:],
                                 start=False, stop=True, skip_group_check=True)
            else:
                nc.tensor.matmul(vp, hA[0:C, ts(t, 128)], BB[:],
                                 start=True, stop=True, skip_group_check=True)
                nc.tensor.matmul(up, hA[0:C + 1, ts(t, 128)], ABc[:],
                                 start=True, stop=True, skip_group_check=True)
            if v_dt[D] == F32:
                vsb = uvsb.tile([128, 256], F32, name="vsb")[:, 0:D]
            else:
                vsb = uvsb.tile([128, 256], BF16, name="vsbh")[:, 0:D]
            nc.scalar.copy(vsb, vp)
            nc.scalar.copy(u[:, t, :], up)
            nc.sync.dma_start(vslice[ts(t, 128), :], vsb)

        def emit_next_prep(t, h, nxt):
            """Transpose h tile t into next layer's hA/hX (+per-tile square)."""
            C2 = nxt["C"]
            tp = tps.tile([128, 128], F32, name="tp", tag="tp")[0:C2, :]
            nc.tensor.transpose(tp, h[:, t, 0:C2], idn_sb[:])
            nc.scalar.activation(nxt["hA"][0:C2, ts(t, 128)], tp, AF.Copy,
                                 scale=1.0)
            nc.scalar.square(nxt["xsq"][0:C2, ts(t, 128)], tp)
            emit_uv(t, nxt["hA"], C2, nxt["D"], nxt["ABc"], nxt["BB"],
                    nxt["u"], nxt["vD"], nxt["l3"])

        def finalize_nsq(nxt):
            C2 = nxt["C"]
            for j in range(JC):
                sqp = tps.tile([1, 512], F32, name="sqp", tag="sqp")
                nc.tensor.matmul(sqp[:], onescol_sb[0:C2, :],
                                 nxt["xsq"][0:C2, ts(j, 512)],
                                 start=True, stop=True)
                nc.scalar.activation(nxt["nsqrow"][0:1, ts(j, 512)], sqp[:],
                                     AF.Copy, scale=-0.5)

        def run_layer(tc, layer, C, D, hA, nsq_t, ones_row, u, h, nxt):
            """Selection + gather + reduce for one EdgeConv layer; interleaves
            next-layer transpose/uv prep two tiles behind the selection."""
            l3 = layer == 3
            vslice = v_drams[D]
            with tc.tile_pool(name=f"L{layer}d", bufs=1, space="PSUM") as dps, \
                 tc.tile_pool(name=f"L{layer}w", bufs=1, space="PSUM") as wps, \
                 tc.tile_pool(name=f"L{layer}dd", bufs=1) as ddp, \
                 tc.tile_pool(name=f"L{layer}sel", bufs=2) as selp, \
                 tc.tile_pool(name=f"L{layer}g", bufs=3) as gp:
                pending = []

                def flush_pending():
                    gb_p, t_p = pending.pop(0)
                    vm = gp.tile([128, D], F32, name="vm")
                    nc.vector.tensor_reduce(out=vm[:],
                                            in_=gb_p.rearrange("p m d -> p d m"),
                                            axis=AX.X, op=ALU.max)
                    nc.vector.tensor_tensor(out=h[:, t_p, :], in0=u[:, t_p, :],
                                            in1=vm[:], op=ALU.add)
                    if nxt is not None:
                        emit_next_prep(t_p, h, nxt)

                for t in range(NT):
                    dp = dps.tile([128, N], F32, name="dp")
                    tchunk = t // 4
                    for j in range(JC):
                        last = (j != tchunk)
                        nc.tensor.matmul(dp[:, ts(j, 512)], hA[0:C, ts(t, 128)],
                                         hA[0:C, ts(j, 512)],
                                         start=True, stop=False)
                        nc.tensor.matmul(dp[:, ts(j, 512)], ones_row,
                                         nsq_t[:, ts(j, 512)],
                                         start=False, stop=last)
                        if not last:
                            nc.tensor.matmul(dp[:, ts(t, 128)], idnN_sb[:],
                                             idnP_sb[:], start=False, stop=True,
                                             skip_group_check=True)
                    dde = ddp.tile([128, N], F32, name="dde")
                    stt_imm(dde[:].bitcast(U32), dp[:].bitcast(U32),
                            0xFFFFF800, iot[:])

                    cand = selp.tile([128, 96], F32, name="cand")
                    for s in range(12):
                        lo = 171 * s
                        hi = min(lo + 171, N)
                        nc.vector.max(out=cand[:, ts(s, 8)],
                                      in_=dde[:, lo:hi])
                    m1 = selp.tile([128, 8], F32, name="m1")
                    m2 = selp.tile([128, 8], F32, name="m2")
                    m3 = selp.tile([128, 8], F32, name="m3")
                    cw = selp.tile([128, 96], F32, name="cw")
                    cw2 = selp.tile([128, 96], F32, name="cw2")
                    nc.vector.max(out=m1[:], in_=cand[:])
                    nc.vector.match_replace(out=cw[:], in_to_replace=m1[:],
                                            in_values=cand[:], imm_value=-1e30)
                    nc.vector.max(out=m2[:], in_=cw[:])
                    nc.vector.match_replace(out=cw2[:], in_to_replace=m2[:],
                                            in_values=cw[:], imm_value=-1e30)
                    nc.vector.max(out=m3[:], in_=cw2[:])

                    jgu = selp.tile([128, 24], U32, name="jgu")
                    ts_imm(jgu[:, 0:8], m1[:].bitcast(U32), 0x7FF)
                    ts_imm(jgu[:, 8:16], m2[:].bitcast(U32), 0x7FF)
                    ts_imm(jgu[:, 16:24], m3[:].bitcast(U32), 0x7FF)
                    jg = selp.tile([128, 20], F16, name="jg")
                    nc.vector.tensor_copy(jg[:], jgu[:, 0:20])

                    wp = wps.tile([128, 8, 20], F32, name="wp")
                    for c in range(8):
                        nc.tensor.matmul(wp[:, c, :], prm[:, c, :], jg[:],
                                         start=True, stop=True)
                    if len(pending) >= 2:
                        flush_pending()
                    wrapped = selp.tile([128, 160], I16, name="wrapped")
                    nc.scalar.copy(
                        wrapped[:].rearrange("q (s c) -> q s c", c=8),
                        wp[:].rearrange("q c s -> q s c"))

                    gb = gp.tile([128, 20, D], v_dt[D], name="gb")
                    q0 = (3 * t) % 4
                    nc.gpsimd.dma_gather(gb[:, 0:8, :], vslice,
                                         wrapped[:, 0:64], 1024, 1024, D,
                                         queue_num=q0)
                    nc.gpsimd.dma_gather(gb[:, 8:16, :], vslice,
                                         wrapped[:, 64:128], 1024, 1024, D,
                                         queue_num=(q0 + 1) % 4)
                    nc.gpsimd.dma_gather(gb[:, 16:20, :], vslice,
                                         wrapped[:, 128:160], 512, 512, D,
                                         queue_num=(q0 + 2) % 4)
                    pending.append((gb, t))
                while pending:
                    flush_pending()
            if nxt is not None:
                finalize_nsq(nxt)

        # ---------------- layer drivers ----------------
        hA3_sb = fp.tile([128, N], F32R)
        xsq_sh = fp.tile([128, N], F32R)
        h3 = fp.tile([128, NT, 256], F32)
        u3 = fp.tile([128, NT, 256], F32)

        with tc.tile_pool(name="lay12", bufs=1) as lp12:
            hA1_sb = lp12.tile([4, N], F32R)
            nc.sync.dma_start(hA1_sb[:], hA1)
            h1 = lp12.tile([128, NT, 64], F32)
            u1 = lp12.tile([128, NT, 64], F32)
            hA2_sb = lp12.tile([65, N], F32R)
            h2 = lp12.tile([128, NT, 128], F32)
            u2 = lp12.tile([128, NT, 128], F32)
            nc.sync.dma_start(hA2_sb[64:65, :], onesr)

            for t in range(NT):
                emit_uv(t, hA1_sb, 3, 64, w["ABc1"], w["BB1"], u1,
                        v_drams[64], False)

            nxt2 = dict(C=64, D=128, hA=hA2_sb,
                        xsq=xsq_sh, ABc=w["ABc2"], BB=w["BB2"], u=u2,
                        vD=v_drams[128], nsqrow=nsq2_sb, l3=False)
            run_layer(tc, 1, 3, 64, hA1_sb, nsq1_sb, onesSB[0:1, :], u1, h1,
                      nxt2)

            nxt3 = dict(C=128, D=256, hA=hA3_sb,
                        xsq=xsq_sh, ABc=w["AB3"], BB=w["BB3"], u=u3,
                        vD=v_drams[256], nsqrow=nsq3, l3=True)
            run_layer(tc, 2, 64, 128, hA2_sb, nsq2_sb, onesSB[32:33, :], u2,
                      h2, nxt3)

        run_layer(tc, 3, 128, 256, hA3_sb, nsq3, onesSB[64:65, :], u3, h3,
                  None)

        # ---------- global max pool + FC head ----------
        with tc.tile_pool(name="head", bufs=1) as hp, \
             tc.tile_pool(name="headps", bufs=1, space="PSUM") as hps:
            gmax = hp.tile([128, 256], F32)
            nc.vector.tensor_reduce(out=gmax[:],
                                    in_=h3.rearrange("p g d -> p d g"),
                                    axis=AX.X, op=ALU.max)
            g0 = hp.tile([128, 1], F32)
            g1 = hp.tile([128, 1], F32)
            for half, gdst in ((0, g0), (1, g1)):
                tp = hps.tile([128, 128], F32, name="tp", tag="tp")
                nc.tensor.transpose(tp[:], gmax[:, ts(half, 128)], idn_sb[:])
                tsb = hp.tile([128, 128], F32, name=f"tsb_{half}")
                nc.scalar.copy(tsb[:], tp[:])
                nc.vector.tensor_reduce(out=gdst[:], in_=tsb[:], axis=AX.X,
                                        op=ALU.max)

            fw1 = [hp.tile([128, 512], F32, name=f"fw1_{kk}") for kk in range(2)]
            fw2 = [hp.tile([128, 256], F32, name=f"fw2_{kk}") for kk in range(4)]
            fw3 = [hp.tile([128, 16], F32, name=f"fw3_{kk}") for kk in range(2)]
            fb1 = hp.tile([128, 4], F32)
            fb2 = hp.tile([128, 2], F32)
            fb3 = hp.tile([16, 1], F32)
            for kk in range(2):
                nc.sync.dma_start(fw1[kk][:], fc1w[ts(kk, 128), :])
                nc.sync.dma_start(fw3[kk][:], fc3w[ts(kk, 128), :])
            for kk in range(4):
                nc.sync.dma_start(fw2[kk][:], fc2w[ts(kk, 128), :])
            nc.sync.dma_start(fb1[:], fc1b)
            nc.sync.dma_start(fb2[:], fc2b)
            nc.sync.dma_start(fb3[:], fc3b)

            a1 = [hp.tile([128, 1], F32, name=f"a1_{m}") for m in range(4)]
            for m in range(4):
                p = hps.tile([128, 1], F32, name="fcp", tag="fcp")
                nc.tensor.matmul(p[:], fw1[0][:, ts(m, 128)], g0[:],
                                 start=True, stop=False)
                nc.tensor.matmul(p[:], fw1[1][:, ts(m, 128)], g1[:],
                                 start=False, stop=True)
                nc.scalar.activation(a1[m][:], p[:], AF.Relu,
                                     bias=fb1[:, m:m + 1], scale=1.0)
            a2 = [hp.tile([128, 1], F32, name=f"a2_{m}") for m in range(2)]
            for m in range(2):
                p = hps.tile([128, 1], F32, name="fcp", tag="fcp")
                for kk in range(4):
                    nc.tensor.matmul(p[:], fw2[kk][:, ts(m, 128)], a1[kk][:],
                                     start=(kk == 0), stop=(kk == 3))
                nc.scalar.activation(a2[m][:], p[:], AF.Relu,
                                     bias=fb2[:, m:m + 1], scale=1.0)
            p3 = hps.tile([128, 1], F32, name="fcp", tag="fcp")[0:16, :]
            for kk in range(2):
                nc.tensor.matmul(p3[:], fw3[kk][:], a2[kk][:],
                                 start=(kk == 0), stop=(kk == 1))
            o_sb = hp.tile([16, 1], F32)
            nc.scalar.activation(o_sb[:], p3[:], AF.Identity, bias=fb3[:],
                                 scale=1.0)
            nc.sync.dma_start(out, o_sb[:])

    nc.finalize()
    return nc


def get_nc():
    if 0 not in _NC_CACHE:
        _NC_CACHE[0] = _builder()
    return _NC_CACHE[0]


def make_in_maps(x, W1, b1, W2, b2, W3, b3, fc1_w, fc1_b, fc2_w, fc2_b,
                 fc3_w, fc3_b):
    f32 = np.float32
    x = np.asarray(x, f32)
    B = x.shape[0]
    W1, W2, W3 = np.asarray(W1, f32), np.asarray(W2, f32), np.asarray(W3, f32)
    prm = np.zeros((128, 8, 128), dtype=np.float16)
    for c in range(8):
        for j in range(128):
            prm[16 * c + (j % 16), c, j] = 1.0
    shared = {
        "ABc1": np.concatenate([W1[:3] - W1[3:6], np.asarray(b1, f32)[None]], 0),
        "BB1": np.ascontiguousarray(W1[3:6]),
        "ABc2": np.concatenate([W2[:64] - W2[64:], np.asarray(b2, f32)[None]], 0),
        "BB2": np.ascontiguousarray(W2[64:]),
        "AB3": np.ascontiguousarray(W3[:128] - W3[128:]),
        "BB3": np.ascontiguousarray(W3[128:]),
        "b3r": np.asarray(b3, f32)[None],
        "idn": np.eye(128, dtype=f32),
        "idnN": (np.eye(128) * -1e30).astype(f32),
        "idnP": np.eye(128, dtype=f32),
        "onesr": np.ones((1, N), f32),
        "onescol": np.ones((128, 1), f32),
        "iot": np.broadcast_to(np.arange(N, dtype=np.uint32), (128, N)).copy(),
        "prm": prm,
        "fc1w": np.asarray(fc1_w, f32),
        "fc1b": np.ascontiguousarray(np.asarray(fc1_b, f32).reshape(4, 128).T),
        "fc2w": np.asarray(fc2_w, f32),
        "fc2b": np.ascontiguousarray(np.asarray(fc2_b, f32).reshape(2, 128).T),
        "fc3w": np.pad(np.asarray(fc3_w, f32), ((0, 0), (0, 6))),
        "fc3b": np.pad(np.asarray(fc3_b, f32), (0, 6))[:, None],
    }
    in_maps = []
    for bb in range(B):
        xb = x[bb]
        xT = np.ascontiguousarray(xb.T)
        nsq = -(xb * xb).sum(-1)[None, :].astype(f32)
        m = dict(shared)
        m["hA1"] = np.concatenate([xT, np.ones((1, N), f32)], 0)
        m["nsq1"] = 0.5 * nsq
        in_maps.append(m)
    return in_maps


def kernel(x, k, W1, b1, W2, b2, W3, b3, fc1_w, fc1_b, fc2_w, fc2_b, fc3_w,
           fc3_b):
    from concourse import bass_utils
    x = np.asarray(x)
    assert int(k) == 20 and x.shape[1] == N and x.shape[2] == 3
    B = x.shape[0]
    assert B == 8
    nc = get_nc()
    in_maps = make_in_maps(x, W1, b1, W2, b2, W3, b3,
                           fc1_w, fc1_b, fc2_w, fc2_b, fc3_w, fc3_b)
    res = bass_utils.run_bass_kernel_spmd(nc, in_maps, core_ids=list(range(B)))
    outs = np.stack([res.results[bb]["out"][:10, 0] for bb in range(B)], axis=0)
    return outs.astype(np.float32)


# revision 26
# speedup vs baseline: 1.6533x; 1.0904x over previous
"""DGCNN (3x DynamicEdgeConv + global max pool + MLP head) on 8 Trainium2
NeuronCores, data-parallel over the batch (one point cloud per core).

EdgeConv algebra: h_ij = [x_i, x_j - x_i] @ W + b = u_i + v_j with
  u = x @ (Wa - Wb) + b,  v = x @ Wb;  out_i = u_i + max_{j in knn(i)} v_j.

kNN key d''_ij = 2 x_i.x_j - |x_j|^2 (largest = nearest); the self column is
killed with a -1e30 diagonal matmul so the top-20 are exactly the neighbors.
Distances use fp32r matmuls; the contraction is augmented so one matmul per
512-col chunk computes 2x.x + nsq (layers 1/2).  Feature rows are stored as
hA = [nsq; feat; ones], hX = [ones; 2*feat] so u/v matmuls slice [feat; ones]
and never touch the late-computed nsq row, letting next-layer transposes and
u/v matmuls interleave into the current layer's selection loop.

The column index is embedded in the low 11 mantissa bits of each distance
(bitwise AND+OR with an iota row) so top-k selection needs no max_index: 16
segment max8's yield 128 candidates, 3x max8 + 2x match_replace pick the
top-24, and indices pop out of the winning values with a bitwise AND.
Neighbor v-rows are fetched with 3 batched dma_gather calls (1024+1024+512
rows) over 4 SWDGE queues; their int16 index list is built by 8 permutation
matmuls that transpose jtab into the gather's wrapped 16-partition layout.
The 20-way neighbor max is one DVE tensor_reduce (deferred 2 tiles to hide
gather latency), added to u to form the layer output.
"""
import numpy as np

_NC_CACHE = {}

N, NT, JC = 2048, 16, 4


def _builder():
    import concourse.bacc as bacc
    import concourse.mybir as mybir
    from concourse.tile import TileContext

    F32 = mybir.dt.float32
    F32R = mybir.dt.float32r
    F16 = mybir.dt.float16
    U32 = mybir.dt.uint32
    I16 = mybir.dt.int16
    AF = mybir.ActivationFunctionType
    ALU = mybir.AluOpType
    AX = mybir.AxisListType

    def ts(i, s):
        return slice(i * s, (i + 1) * s)

    nc = bacc.Bacc("TRN2", num_devices=8, num_swdge_queues=4)

    def din(name, shape, dt=F32R):
        return nc.dram_tensor(name, shape, dt, kind="ExternalInput").ap()

    hA1 = din("hA1", [4, N])            # [x^T; ones]
    nsq1 = din("nsq1", [1, N])          # -0.5 |x|^2
    ABc1 = din("ABc1", [4, 64])         # [W1a-W1b; b1]
    BB1 = din("BB1", [3, 64])
    ABc2 = din("ABc2", [65, 128])       # [W2a-W2b; b2]
    BB2 = din("BB2", [64, 128])
    AB3 = din("AB3", [128, 256])
    BB3 = din("BB3", [128, 256])
    b3r = din("b3r", [1, 256])
    idn = din("idn", [128, 128], F32)
    idnN = din("idnN", [128, 128])      # -1e30 * I
    idnP = din("idnP", [128, 128])      # I
    onesr = din("onesr", [1, N])        # ones row
    onescol = din("onescol", [128, 1])  # ones column
    iot_in = din("iot", [128, N], U32)
    prm_in = din("prm", [128, 8, 128], F16)
    fc1w = din("fc1w", [256, 512], F32)
    fc1b = din("fc1b", [128, 4], F32)
    fc2w = din("fc2w", [512, 256], F32)
    fc2b = din("fc2b", [128, 2], F32)
    fc3w = din("fc3w", [256, 16], F32)
    fc3b = din("fc3b", [16, 1], F32)
    out = nc.dram_tensor("out", [16, 1], F32, kind="ExternalOutput").ap()

    def ts_imm(out_ap, in0, imm):
        eng = nc.vector
        return eng.add_instruction(
            mybir.InstTensorScalarPtr(
                name=eng.bass.get_next_instruction_name(),
                op0=ALU.bitwise_and, op1=ALU.bypass,
                ins=[eng.lower_ap(in0),
                     mybir.ImmediateValue(dtype=U32, value=imm)],
                outs=[eng.lower_ap(out_ap)]))

    def stt_imm(out_ap, in0, imm, in1):
        eng = nc.vector
        return eng.add_instruction(
            mybir.InstTensorScalarPtr(
                name=eng.bass.get_next_instruction_name(),
                is_scalar_tensor_tensor=True,
                op0=ALU.bitwise_and, op1=ALU.bitwise_or,
                ins=[eng.lower_ap(in0),
                     mybir.ImmediateValue(dtype=U32, value=imm),
                     eng.lower_ap(in1)],
                outs=[eng.lower_ap(out_ap)]))

    from contextlib import ExitStack
    with TileContext(nc) as tc, ExitStack() as stack:
        cp = stack.enter_context(tc.tile_pool(name="const", bufs=1))
        fp = stack.enter_context(tc.tile_pool(name="feat", bufs=1))
        vdp = stack.enter_context(tc.tile_pool(name="vdram", bufs=1, space="DRAM"))
        uvps = stack.enter_context(tc.tile_pool(name="uvps", bufs=1, space="PSUM"))
        tps = stack.enter_context(tc.tile_pool(name="tps", bufs=1, space="PSUM"))
        uvsb = stack.enter_context(tc.tile_pool(name="uvsb", bufs=4))

        v_drams = {}
        BF16 = mybir.dt.bfloat16
        v_dt = {64: F32, 128: BF16, 256: BF16}
        for _D in (64, 128, 256):
            v_drams[_D] = vdp.tile([N, _D], v_dt[_D], name=f"v_dram{_D}")
        idn_sb = cp.tile([128, 128], F32)
        nc.sync.dma_start(idn_sb[:], idn)
        idnN_sb = cp.tile([128, 128], F32R)
        nc.sync.dma_start(idnN_sb[:], idnN)
        idnP_sb = cp.tile([128, 128], F32R)
        nc.sync.dma_start(idnP_sb[:], idnP)
        onesSB = cp.tile([65, 128], F32R)
        nc.sync.dma_start(onesSB[0:1, :], onesr[0:1, 0:128])
        nc.sync.dma_start(onesSB[32:33, :], onesr[0:1, 0:128])
        nc.sync.dma_start(onesSB[64:65, :], onesr[0:1, 0:128])
        nsqall = cp.tile([65, N], F32R)
        onescol_sb = cp.tile([128, 1], F32R)
        nc.sync.dma_start(onescol_sb[:], onescol)
        iot = cp.tile([128, N], U32)
        nc.sync.dma_start(iot[:], iot_in)
        prm = cp.tile([128, 8, 128], F16)
        nc.sync.dma_start(prm[:], prm_in)
        ones1 = onesSB[0:1, 0:128]
        nsq1_sb = nsqall[0:1, :]
        nsq2_sb = nsqall[32:33, :]
        nsq3 = nsqall[64:65, :]
        nc.sync.dma_start(nsq1_sb, nsq1)

        w = {}
        for nm, ap_, shape in [("ABc1", ABc1, [4, 64]), ("BB1", BB1, [3, 64]),
                               ("ABc2", ABc2, [65, 128]), ("BB2", BB2, [64, 128]),
                               ("AB3", AB3, [128, 256]), ("BB3", BB3, [128, 256]),
                               ("b3r", b3r, [1, 256])]:
            t_ = cp.tile(shape, F32R, name=f"w_{nm}")
            nc.sync.dma_start(t_[:], ap_)
            w[nm] = t_

        def emit_uv(t, hA, C, D, ABc, BB, u, vslice, l3):
            """u/v matmuls for tile t; hA rows [nsq(0); feat(1..C); ones(C+1)]
            (L3: hA = feat only, bias via ones1 @ b3r)."""
            pair = uvps.tile([128, 512], F32, name="uvpair")
            up = pair[:, 0:D]
            vp = pair[:, 256:256 + D]
            if l3:
                nc.tensor.matmul(vp, hA[:, ts(t, 128)], BB[:],
                                 start=True, stop=True, skip_group_check=True)
                nc.tensor.matmul(up, hA[:, ts(t, 128)], ABc[:],
                                 start=True, stop=False, skip_group_check=True)
                nc.tensor.matmul(up, ones1, w["b3r"][:],
                                 start=False, stop=True, skip_group_check=True)
            else:
                nc.tensor.matmul(vp, hA[0:C, ts(t, 128)], BB[:],
                                 start=True, stop=True, skip_group_check=True)
                nc.tensor.matmul(up, hA[0:C + 1, ts(t, 128)], ABc[:],
                                 start=True, stop=True, skip_group_check=True)
            if v_dt[D] == F32:
                vsb = uvsb.tile([128, 256], F32, name="vsb")[:, 0:D]
            else:
                vsb = uvsb.tile([128, 256], BF16, name="vsbh")[:, 0:D]
            nc.scalar.copy(vsb, vp)
            nc.scalar.copy(u[:, t, :], up)
            nc.sync.dma_start(vslice[ts(t, 128), :], vsb)

        def emit_next_prep(t, h, nxt):
            """Transpose h tile t into next layer's hA/hX (+per-tile square)."""
            C2 = nxt["C"]
            tp = tps.tile([128, 128], F32, name="tp", tag="tp")[0:C2, :]
            nc.tensor.transpose(tp, h[:, t, 0:C2], idn_sb[:])
            nc.scalar.activation(nxt["hA"][0:C2, ts(t, 128)], tp, AF.Copy,
                                 scale=1.0)
            nc.scalar.square(nxt["xsq"][0:C2, ts(t, 128)], tp)
            emit_uv(t, nxt["hA"], C2, nxt["D"], nxt["ABc"], nxt["BB"],
                    nxt["u"], nxt["vD"], nxt["l3"])

        def finalize_nsq(nxt):
            C2 = nxt["C"]
            for j in range(JC):
                sqp = tps.tile([1, 512], F32, name="sqp", tag="sqp")
                nc.tensor.matmul(sqp[:], onescol_sb[0:C2, :],
                                 nxt["xsq"][0:C2, ts(j, 512)],
                                 start=True, stop=True)
                nc.scalar.activation(nxt["nsqrow"][0:1, ts(j, 512)], sqp[:],
                                     AF.Copy, scale=-0.5)

        def run_layer(tc, layer, C, D, hA, nsq_t, ones_row, u, h, nxt):
            """Selection + gather + reduce for one EdgeConv layer; interleaves
            next-layer transpose/uv prep two tiles behind the selection."""
            l3 = layer == 3
            vslice = v_drams[D]
            with tc.tile_pool(name=f"L{layer}d", bufs=1, space="PSUM") as dps, \
                 tc.tile_pool(name=f"L{layer}w", bufs=1, space="PSUM") as wps, \
                 tc.tile_pool(name=f"L{layer}dd", bufs=1) as ddp, \
                 tc.tile_pool(name=f"L{layer}sel", bufs=2) as selp, \
                 tc.tile_pool(name=f"L{layer}g", bufs=3) as gp:
                pending = []

                def flush_pending():
                    gb_p, t_p = pending.pop(0)
                    vm = gp.tile([128, D], v_dt[D], name="vm")
                    if v_dt[D] == F32:
                        nc.vector.tensor_reduce(out=vm[:],
                                                in_=gb_p.rearrange("p m d -> p d m"),
                                                axis=AX.X, op=ALU.max)
                    else:
                        t1 = gp.tile([128, 10, D], BF16, name="t1")
                        nc.vector.tensor_tensor(out=t1[:], in0=gb_p[:, 0:10, :],
                                                in1=gb_p[:, 10:20, :], op=ALU.max)
                        t2 = gp.tile([128, 5, D], BF16, name="t2")
                        nc.vector.tensor_tensor(out=t2[:], in0=t1[:, 0:5, :],
                                                in1=t1[:, 5:10, :], op=ALU.max)
                        nc.vector.tensor_reduce(out=vm[:],
                                                in_=t2.rearrange("p m d -> p d m"),
                                                axis=AX.X, op=ALU.max)
                    nc.vector.tensor_tensor(out=h[:, t_p, :], in0=u[:, t_p, :],
                                            in1=vm[:], op=ALU.add)
                    if nxt is not None:
                        emit_next_prep(t_p, h, nxt)

                for t in range(NT):
                    dp = dps.tile([128, N], F32, name="dp")
                    tchunk = t // 4
                    for j in range(JC):
                        last = (j != tchunk)
                        nc.tensor.matmul(dp[:, ts(j, 512)], hA[0:C, ts(t, 128)],
                                         hA[0:C, ts(j, 512)],
                                         start=True, stop=False)
                        nc.tensor.matmul(dp[:, ts(j, 512)], ones_row,
                                         nsq_t[:, ts(j, 512)],
                                         start=False, stop=last)
                        if not last:
                            nc.tensor.matmul(dp[:, ts(t, 128)], idnN_sb[:],
                                             idnP_sb[:], start=False, stop=True,
                                             skip_group_check=True)
                    dde = ddp.tile([128, N], F32, name="dde")
                    stt_imm(dde[:].bitcast(U32), dp[:].bitcast(U32),
                            0xFFFFF800, iot[:])

                    cand = selp.tile([128, 96], F32, name="cand")
                    for s in range(12):
                        lo = 171 * s
                        hi = min(lo + 171, N)
                        nc.vector.max(out=cand[:, ts(s, 8)],
                                      in_=dde[:, lo:hi])
                    m1 = selp.tile([128, 8], F32, name="m1")
                    m2 = selp.tile([128, 8], F32, name="m2")
                    m3 = selp.tile([128, 8], F32, name="m3")
                    cw = selp.tile([128, 96], F32, name="cw")
                    cw2 = selp.tile([128, 96], F32, name="cw2")
                    nc.vector.max(out=m1[:], in_=cand[:])
                    nc.vector.match_replace(out=cw[:], in_to_replace=m1[:],
                                            in_values=cand[:], imm_value=-1e30)
                    nc.vector.max(out=m2[:], in_=cw[:])
                    nc.vector.match_replace(out=cw2[:], in_to_replace=m2[:],
                                            in_values=cw[:], imm_value=-1e30)
                    nc.vector.max(out=m3[:], in_=cw2[:])

                    jgu = selp.tile([128, 24], U32, name="jgu")
                    ts_imm(jgu[:, 0:8], m1[:].bitcast(U32), 0x7FF)
                    ts_imm(jgu[:, 8:16], m2[:].bitcast(U32), 0x7FF)
                    ts_imm(jgu[:, 16:24], m3[:].bitcast(U32), 0x7FF)
                    jg = selp.tile([128, 20], F16, name="jg")
                    nc.vector.tensor_copy(jg[:], jgu[:, 0:20])

                    wp = wps.tile([128, 8, 20], F32, name="wp")
                    for c in range(8):
                        nc.tensor.matmul(wp[:, c, :], prm[:, c, :], jg[:],
                                         start=True, stop=True)
                    if len(pending) >= 2:
                        flush_pending()
                    wrapped = selp.tile([128, 160], I16, name="wrapped")
                    nc.scalar.copy(
                        wrapped[:].rearrange("q (s c) -> q s c", c=8),
                        wp[:].rearrange("q c s -> q s c"))

                    gb = gp.tile([128, 20, D], v_dt[D], name="gb")
                    q0 = (3 * t) % 4
                    nc.gpsimd.dma_gather(gb[:, 0:8, :], vslice,
                                         wrapped[:, 0:64], 1024, 1024, D,
                                         queue_num=q0)
                    nc.gpsimd.dma_gather(gb[:, 8:16, :], vslice,
                                         wrapped[:, 64:128], 1024, 1024, D,
                                         queue_num=(q0 + 1) % 4)
                    nc.gpsimd.dma_gather(gb[:, 16:20, :], vslice,
                                         wrapped[:, 128:160], 512, 512, D,
                                         queue_num=(q0 + 2) % 4)
                    pending.append((gb, t))
                while pending:
                    flush_pending()
            if nxt is not None:
                finalize_nsq(nxt)

        # ---------------- layer drivers ----------------
        hA3_sb = fp.tile([128, N], F32R)
        xsq_sh = fp.tile([128, N], F32R)
        h3 = fp.tile([128, NT, 256], F32)
        u3 = fp.tile([128, NT, 256], F32)

        with tc.tile_pool(name="lay12", bufs=1) as lp12:
            hA1_sb = lp12.tile([4, N], F32R)
            nc.sync.dma_start(hA1_sb[:], hA1)
            h1 = lp12.tile([128, NT, 64], F32)
            u1 = lp12.tile([128, NT, 64], F32)
            hA2_sb = lp12.tile([65, N], F32R)
            h2 = lp12.tile([128, NT, 128], F32)
            u2 = lp12.tile([128, NT, 128], F32)
            nc.sync.dma_start(hA2_sb[64:65, :], onesr)

            for t in range(NT):
                emit_uv(t, hA1_sb, 3, 64, w["ABc1"], w["BB1"], u1,
                        v_drams[64], False)

            nxt2 = dict(C=64, D=128, hA=hA2_sb,
                        xsq=xsq_sh, ABc=w["ABc2"], BB=w["BB2"], u=u2,
                        vD=v_drams[128], nsqrow=nsq2_sb, l3=False)
            run_layer(tc, 1, 3, 64, hA1_sb, nsq1_sb, onesSB[0:1, :], u1, h1,
                      nxt2)

            nxt3 = dict(C=128, D=256, hA=hA3_sb,
                        xsq=xsq_sh, ABc=w["AB3"], BB=w["BB3"], u=u3,
                        vD=v_drams[256], nsqrow=nsq3, l3=True)
            run_layer(tc, 2, 64, 128, hA2_sb, nsq2_sb, onesSB[32:33, :], u2,
                      h2, nxt3)

        run_layer(tc, 3, 128, 256, hA3_sb, nsq3, onesSB[64:65, :], u3, h3,
                  None)

        # ---------- global max pool + FC head ----------
        with tc.tile_pool(name="head", bufs=1) as hp, \
             tc.tile_pool(name="headps", bufs=1, space="PSUM") as hps:
            gmax = hp.tile([128, 256], F32)
            nc.vector.tensor_reduce(out=gmax[:],
                                    in_=h3.rearrange("p g d -> p d g"),
                                    axis=AX.X, op=ALU.max)
            g0 = hp.tile([128, 1], F32)
            g1 = hp.tile([128, 1], F32)
            for half, gdst in ((0, g0), (1, g1)):
                tp = hps.tile([128, 128], F32, name="tp", tag="tp")
                nc.tensor.transpose(tp[:], gmax[:, ts(half, 128)], idn_sb[:])
                tsb = hp.tile([128, 128], F32, name=f"tsb_{half}")
                nc.scalar.copy(tsb[:], tp[:])
                nc.vector.tensor_reduce(out=gdst[:], in_=tsb[:], axis=AX.X,
                                        op=ALU.max)

            fw1 = [hp.tile([128, 512], F32, name=f"fw1_{kk}") for kk in range(2)]
            fw2 = [hp.tile([128, 256], F32, name=f"fw2_{kk}") for kk in range(4)]
            fw3 = [hp.tile([128, 16], F32, name=f"fw3_{kk}") for kk in range(2)]
            fb1 = hp.tile([128, 4], F32)
            fb2 = hp.tile([128, 2], F32)
            fb3 = hp.tile([16, 1], F32)
            for kk in range(2):
                nc.sync.dma_start(fw1[kk][:], fc1w[ts(kk, 128), :])
                nc.sync.dma_start(fw3[kk][:], fc3w[ts(kk, 128), :])
            for kk in range(4):
                nc.sync.dma_start(fw2[kk][:], fc2w[ts(kk, 128), :])
            nc.sync.dma_start(fb1[:], fc1b)
            nc.sync.dma_start(fb2[:], fc2b)
            nc.sync.dma_start(fb3[:], fc3b)

            a1 = [hp.tile([128, 1], F32, name=f"a1_{m}") for m in range(4)]
            for m in range(4):
                p = hps.tile([128, 1], F32, name="fcp", tag="fcp")
                nc.tensor.matmul(p[:], fw1[0][:, ts(m, 128)], g0[:],
                                 start=True, stop=False)
                nc.tensor.matmul(p[:], fw1[1][:, ts(m, 128)], g1[:],
                                 start=False, stop=True)
                nc.scalar.activation(a1[m][:], p[:], AF.Relu,
                                     bias=fb1[:, m:m + 1], scale=1.0)
            a2 = [hp.tile([128, 1], F32, name=f"a2_{m}") for m in range(2)]
            for m in range(2):
                p = hps.tile([128, 1], F32, name="fcp", tag="fcp")
                for kk in range(4):
                    nc.tensor.matmul(p[:], fw2[kk][:, ts(m, 128)], a1[kk][:],
                                     start=(kk == 0), stop=(kk == 3))
                nc.scalar.activation(a2[m][:], p[:], AF.Relu,
                                     bias=fb2[:, m:m + 1], scale=1.0)
            p3 = hps.tile([128, 1], F32, name="fcp", tag="fcp")[0:16, :]
            for kk in range(2):
                nc.tensor.matmul(p3[:], fw3[kk][:], a2[kk][:],
                                 start=(kk == 0), stop=(kk == 1))
            o_sb = hp.tile([16, 1], F32)
            nc.scalar.activation(o_sb[:], p3[:], AF.Identity, bias=fb3[:],
                                 scale=1.0)
            nc.sync.dma_start(out, o_sb[:])

    nc.finalize()
    return nc


def get_nc():
    if 0 not in _NC_CACHE:
        _NC_CACHE[0] = _builder()
    return _NC_CACHE[0]


def make_in_maps(x, W1, b1, W2, b2, W3, b3, fc1_w, fc1_b, fc2_w, fc2_b,
                 fc3_w, fc3_b):
    f32 = np.float32
    x = np.asarray(x, f32)
    B = x.shape[0]
    W1, W2, W3 = np.asarray(W1, f32), np.asarray(W2, f32), np.asarray(W3, f32)
    prm = np.zeros((128, 8, 128), dtype=np.float16)
    for c in range(8):
        for j in range(128):
            prm[16 * c + (j % 16), c, j] = 1.0
    shared = {
        "ABc1": np.concatenate([W1[:3] - W1[3:6], np.asarray(b1, f32)[None]], 0),
        "BB1": np.ascontiguousarray(W1[3:6]),
        "ABc2": np.concatenate([W2[:64] - W2[64:], np.asarray(b2, f32)[None]], 0),
        "BB2": np.ascontiguousarray(W2[64:]),
        "AB3": np.ascontiguousarray(W3[:128] - W3[128:]),
        "BB3": np.ascontiguousarray(W3[128:]),
        "b3r": np.asarray(b3, f32)[None],
        "idn": np.eye(128, dtype=f32),
        "idnN": (np.eye(128) * -1e30).astype(f32),
        "idnP": np.eye(128, dtype=f32),
        "onesr": np.ones((1, N), f32),
        "onescol": np.ones((128, 1), f32),
        "iot": np.broadcast_to(np.arange(N, dtype=np.uint32), (128, N)).copy(),
        "prm": prm,
        "fc1w": np.asarray(fc1_w, f32),
        "fc1b": np.ascontiguousarray(np.asarray(fc1_b, f32).reshape(4, 128).T),
        "fc2w": np.asarray(fc2_w, f32),
        "fc2b": np.ascontiguousarray(np.asarray(fc2_b, f32).reshape(2, 128).T),
        "fc3w": np.pad(np.asarray(fc3_w, f32), ((0, 0), (0, 6))),
        "fc3b": np.pad(np.asarray(fc3_b, f32), (0, 6))[:, None],
    }
    in_maps = []
    for bb in range(B):
        xb = x[bb]
        xT = np.ascontiguousarray(xb.T)
        nsq = -(xb * xb).sum(-1)[None, :].astype(f32)
        m = dict(shared)
        m["hA1"] = np.concatenate([xT, np.ones((1, N), f32)], 0)
        m["nsq1"] = 0.5 * nsq
        in_maps.append(m)
    return in_maps


def kernel(x, k, W1, b1, W2, b2, W3, b3, fc1_w, fc1_b, fc2_w, fc2_b, fc3_w,
           fc3_b):
    from concourse import bass_utils
    x = np.asarray(x)
    assert int(k) == 20 and x.shape[1] == N and x.shape[2] == 3
    B = x.shape[0]
    assert B == 8
    nc = get_nc()
    in_maps = make_in_maps(x, W1, b1, W2, b2, W3, b3,
                           fc1_w, fc1_b, fc2_w, fc2_b, fc3_w, fc3_b)
    res = bass_utils.run_bass_kernel_spmd(nc, in_maps, core_ids=list(range(B)))
    outs = np.stack([res.results[bb]["out"][:10, 0] for bb in range(B)], axis=0)
    return outs.astype(np.float32)
